# revision 1
# baseline (speedup 1.0000x reference)
"""GTE contrastive loss kernel for 8 Trainium2 NeuronCores.

Math (reference): loss = -mean_i( cos(a_i,p_i)/T - log(partition_i) ),
partition_i = sum_j E_ap[i,j] + sum_j E_aa[i,j] + sum_j E_ap[j,i]
            + sum_j E_pp[j,i] - 2*exp(1/T),   E_xy = exp(cos/T).

Sharding: core k owns row block k (1024 rows).  Inputs are rotated by
-1024k rows per core so one SPMD program suffices: "my rows" are always
rows 0:1024 of the rotated input, and column block j means global block
(k+j) mod 8.  Symmetry of E_aa/E_pp lets each core compute only column
blocks 0..4: blocks 1..3 also emit column sums which cover the missing
row-sum pieces of blocks 5..7 on other cores; block 4 is computed by
both endpoint cores (rowsum only) so it is never double counted.
"""

import os
import sys

import numpy as np

for _p in ("/opt/trn_rl_repo", os.path.expanduser("/root/.axon_site/_ro/trn_rl_repo")):
    if os.path.isdir(_p) and _p not in sys.path:
        sys.path.insert(0, _p)

from concourse import bass, masks, tile  # noqa: E402
from concourse.bass_utils import run_bass_kernel_spmd  # noqa: E402

mybir = bass.mybir
F32 = mybir.dt.float32

N, D, NCORES = 8192, 64, 8
B = N // NCORES            # 1024 rows per core
MT = B // 128              # 8 row tiles of 128
TFULL = N // 128           # 64 transpose tiles
INV_T = 20.0

AP_BLOCKS = list(range(8))       # ap: all column blocks, all with colsum
SYM_BLOCKS = [0, 1, 2, 3, 4]     # aa/pp: cyclic half
SYM_COL_BLOCKS = [1, 2, 3]       # aa/pp blocks that also emit colsums


def _emit_load_normalize(nc, tc, ctx, dram_in, name):
    """DMA [N,D] -> [128,64,64] nat layout and row-normalize in place."""
    singles = ctx.enter_context(tc.tile_pool(name=f"{name}_sb", bufs=1))
    tmp = ctx.enter_context(tc.tile_pool(name=f"{name}_tmp", bufs=1))

    nat = singles.tile([128, TFULL, D], F32)
    sq = tmp.tile([128, TFULL, D], F32, tag="sqtmp")
    src = dram_in[:].rearrange("(t p) d -> p t d", p=128)
    for h in range(4):
        t0, t1 = h * (TFULL // 4), (h + 1) * (TFULL // 4)
        nc.sync.dma_start(out=nat[:, t0:t1, :], in_=src[:, t0:t1, :])
        nc.vector.tensor_mul(sq[:, t0:t1, :], nat[:, t0:t1, :], nat[:, t0:t1, :])
    ss = singles.tile([128, TFULL], F32)
    nc.vector.tensor_reduce(ss[:], sq[:], axis=mybir.AxisListType.X,
                            op=mybir.AluOpType.add)
    nrm = singles.tile([128, TFULL], F32)
    nc.scalar.activation(nrm[:], ss[:], mybir.ActivationFunctionType.Sqrt)
    inv = singles.tile([128, TFULL], F32)
    nc.vector.reciprocal(inv[:], nrm[:])
    # nat[p, t, d] *= inv[p, t]  (broadcast along d)
    inv_b = inv[:].broadcast_to([128, TFULL, D])
    nat2 = singles.tile([128, TFULL, D], F32, tag=f"{name}_nat2")
    nc.vector.tensor_mul(nat2[:], nat[:], inv_b)
    return singles, nat2


def _emit_transpose(nc, tc, singles, nat, ident, name):
    """PE-transpose normalized nat [128,64,64] -> xT [64, N]."""
    xT = singles.tile([64, N], F32, tag=f"{name}_xT")
    with tc.tile_pool(name=f"{name}_tr", bufs=2, space="PSUM") as trp:
        for q in range(TFULL // 4):
            tr = trp.tile([64, 512], F32, tag="tr")
            for s in range(4):
                t = q * 4 + s
                nc.tensor.transpose(tr[:, s * 128:(s + 1) * 128], nat[:, t, :],
                                    ident[:])
            nc.vector.tensor_copy(xT[:, q * 512:(q + 1) * 512], tr[:])
    return xT


def build_program():
    nc = bass.Bass()
    a_in = nc.declare_dram_parameter("a", [N, D], F32, isOutput=False)
    p_in = nc.declare_dram_parameter("p", [N, D], F32, isOutput=False)
    o_st_ap = nc.declare_dram_parameter("st_ap", [128, MT * 8], F32, isOutput=True)
    o_st_aa = nc.declare_dram_parameter("st_aa", [128, MT * 5], F32, isOutput=True)
    o_st_pp = nc.declare_dram_parameter("st_pp", [128, MT * 5], F32, isOutput=True)
    o_diag = nc.declare_dram_parameter("diag", [128, MT], F32, isOutput=True)
    o_cs_ap = nc.declare_dram_parameter("cs_ap", [1, 8 * B], F32, isOutput=True)
    o_cs_aa = nc.declare_dram_parameter("cs_aa", [1, 3 * B], F32, isOutput=True)
    o_cs_pp = nc.declare_dram_parameter("cs_pp", [1, 3 * B], F32, isOutput=True)

    with tile.TileContext(nc) as tc:
        import contextlib

        with contextlib.ExitStack() as ctx:
            a_sing, a_nat = _emit_load_normalize(nc, tc, ctx, a_in, "a")
            p_sing, p_nat = _emit_load_normalize(nc, tc, ctx, p_in, "p")

            res = ctx.enter_context(tc.tile_pool(name="results", bufs=1))
            st_ap = res.tile([128, MT * 8], F32)
            st_aa = res.tile([128, MT * 5], F32)
            st_pp = res.tile([128, MT * 5], F32)
            diag = res.tile([128, MT], F32)
            ones_t = res.tile([128, 128], F32)
            nc.vector.memset(ones_t[:], 1.0)

            # diagonal cos(a_i, p_i) for own rows (block 0 of rotated input)
            dtmp = res.tile([128, MT, D], F32)
            nc.vector.tensor_mul(dtmp[:], a_nat[:, 0:MT, :], p_nat[:, 0:MT, :])
            nc.vector.tensor_reduce(diag[:], dtmp[:], axis=mybir.AxisListType.X,
                                    op=mybir.AluOpType.add)

            ident0 = res.tile([128, 128], F32)
            masks.make_identity(nc, ident0[:])
            ident = res.tile([128, 128], F32)
            nc.vector.tensor_copy(ident[:], ident0[:])
            aT = _emit_transpose(nc, tc, a_sing, a_nat, ident, "a")
            pT = _emit_transpose(nc, tc, p_sing, p_nat, ident, "p")

            csp = ctx.enter_context(tc.tile_pool(name="csstage", bufs=2))
            mmp = ctx.enter_context(tc.tile_pool(name="mm", bufs=2, space="PSUM"))
            colp = ctx.enter_context(tc.tile_pool(name="col", bufs=2, space="PSUM"))
            ep = ctx.enter_context(tc.tile_pool(name="etile", bufs=3))

            jobs = []
            for g, j in enumerate(AP_BLOCKS):
                jobs.append((aT, pT, j, st_ap, g, 8, o_cs_ap, j))
            for g, j in enumerate(SYM_BLOCKS):
                cj = SYM_COL_BLOCKS.index(j) if j in SYM_COL_BLOCKS else None
                jobs.append((aT, aT, j, st_aa, g, 5, o_cs_aa, cj))
            for g, j in enumerate(SYM_BLOCKS):
                cj = SYM_COL_BLOCKS.index(j) if j in SYM_COL_BLOCKS else None
                jobs.append((pT, pT, j, st_pp, g, 5, o_cs_pp, cj))

            for xT, yT, j, st, g, ng, cs, cj in jobs:
                col_ps = None
                if cj is not None:
                    col_ps = colp.tile([128, B], F32, tag="col")
                for m in range(MT):
                    mm_ps = mmp.tile([128, B], F32, tag="mm")
                    for c in range(2):
                        nc.tensor.matmul(
                            mm_ps[:, c * 512:(c + 1) * 512],
                            xT[:, m * 128:(m + 1) * 128],
                            yT[:, j * B + c * 512: j * B + (c + 1) * 512],
                            start=True, stop=True,
                        )
                    e = ep.tile([128, B], F32, tag="e")
                    nc.scalar.activation(
                        e[:], mm_ps[:], mybir.ActivationFunctionType.Exp,
                        scale=INV_T,
                        accum_out=st[:, m * ng + g: m * ng + g + 1],
                    )
                    if col_ps is not None:
                        for c in range(2):
                            nc.tensor.matmul(
                                col_ps[:, c * 512:(c + 1) * 512],
                                ones_t[:],
                                e[:, c * 512:(c + 1) * 512],
                                start=(m == 0), stop=(m == MT - 1),
                                skip_group_check=True,
                            )
                if col_ps is not None:
                    cstage = csp.tile([1, B], F32, tag="cs")
                    nc.scalar.activation(cstage[:], col_ps[0:1, :],
                                         mybir.ActivationFunctionType.Copy)
                    nc.sync.dma_start(out=cs[0:1, cj * B:(cj + 1) * B],
                                      in_=cstage[:])

            nc.sync.dma_start(out=o_st_ap[:], in_=st_ap[:])
            nc.sync.dma_start(out=o_st_aa[:], in_=st_aa[:])
            nc.sync.dma_start(out=o_st_pp[:], in_=st_pp[:])
            nc.sync.dma_start(out=o_diag[:], in_=diag[:])
    return nc


def combine(core_outs):
    """core_outs: list (per core) of dicts with the 7 output arrays."""
    self_term = np.exp(np.float32(INV_T))
    rs_ap = np.empty(N, np.float32)
    rs_aa = np.empty(N, np.float32)
    rs_pp = np.empty(N, np.float32)
    diag = np.empty(N, np.float32)
    cs_ap_tot = np.zeros(N, np.float64)
    aa_contrib = np.zeros(N, np.float64)
    pp_contrib = np.zeros(N, np.float64)

    for k, o in enumerate(core_outs):
        sl = slice(k * B, (k + 1) * B)
        # st[p, m*ng+g] -> local row m*128+p; sum over g
        rs_ap[sl] = o["st_ap"].reshape(128, MT, 8).sum(-1).T.reshape(B)
        rs_aa[sl] = o["st_aa"].reshape(128, MT, 5).sum(-1).T.reshape(B)
        rs_pp[sl] = o["st_pp"].reshape(128, MT, 5).sum(-1).T.reshape(B)
        diag[sl] = o["diag"].T.reshape(B)

        cs_ap_tot += np.roll(o["cs_ap"].reshape(N).astype(np.float64), k * B)
        for row, j in enumerate(SYM_COL_BLOCKS):
            v = np.zeros(N, np.float64)
            v[j * B:(j + 1) * B] = o["cs_aa"].reshape(3, B)[row]
            aa_contrib += np.roll(v, k * B)
            v = np.zeros(N, np.float64)
            v[j * B:(j + 1) * B] = o["cs_pp"].reshape(3, B)[row]
            pp_contrib += np.roll(v, k * B)

    partition = (rs_ap.astype(np.float64) + cs_ap_tot
                 + rs_aa.astype(np.float64) + aa_contrib
                 + rs_pp.astype(np.float64) + pp_contrib
                 - 2.0 * float(self_term))
    pos_logit = INV_T * diag.astype(np.float64)
    loss = -(pos_logit - np.log(partition)).mean()
    return np.float32(loss)


def _split_waits(nc):
    """Walrus codegen allows ~1 sync wait per instruction; hoist extra
    waits onto same-engine NoOps inserted just before the instruction."""
    for fn in nc.m.functions:
        for blk in fn.blocks:
            new = []
            for inst in blk.instructions:
                si = getattr(inst, "sync_info", None)
                keep = 1
                if si is not None and si.on_wait and len(si.on_wait) > keep:
                    waits = list(si.on_wait)
                    for i, w in enumerate(waits[:-keep]):
                        nop = mybir.InstNoOp(name=f"{inst.name}-sw{i}")
                        nop.engine = inst.engine
                        nop.sync_info = mybir.SyncInfo(on_wait=[w], on_update=[])
                        new.append(nop)
                    inst.sync_info = mybir.SyncInfo(
                        on_wait=list(waits[-keep:]),
                        on_update=list(si.on_update))
                new.append(inst)
            blk.instructions = new


_NC_CACHE = None


def _get_program():
    global _NC_CACHE
    if _NC_CACHE is None:
        _NC_CACHE = build_program()
        _split_waits(_NC_CACHE)
    return _NC_CACHE


def run(anchor_embeddings, positive_embeddings, trace=False, **trace_kwargs):
    a = np.ascontiguousarray(anchor_embeddings, dtype=np.float32)
    p = np.ascontiguousarray(positive_embeddings, dtype=np.float32)
    in_maps = [
        {"a": np.roll(a, -k * B, axis=0), "p": np.roll(p, -k * B, axis=0)}
        for k in range(NCORES)
    ]
    nc = _get_program()
    res = run_bass_kernel_spmd(nc, in_maps, list(range(NCORES)), trace=trace,
                               **trace_kwargs)
    return combine(res.results), res


def kernel(anchor_embeddings, positive_embeddings):
    loss, _ = run(anchor_embeddings, positive_embeddings)
    return loss



# revision 13
# speedup vs baseline: 1.5027x; 1.5027x over previous
"""GTE contrastive loss kernel for 8 Trainium2 NeuronCores.

Math (reference): loss = -mean_i( cos(a_i,p_i)/T - log(partition_i) ),
partition_i = sum_j E_ap[i,j] + sum_j E_aa[i,j] + sum_j E_ap[j,i]
            + sum_j E_pp[j,i] - 2*exp(1/T),   E_xy = exp(cos/T).

Sharding: core k owns row block k (1024 rows).  Inputs are rotated by
-1024k rows per core so one SPMD program suffices: "my rows" are always
rows 0:1024 of the rotated input, and column block j means global block
(k+j) mod 8.  Symmetry of E_aa/E_pp lets each core compute only column
blocks 0..4: blocks 1..3 also emit column sums which cover the missing
row-sum pieces of blocks 5..7 on other cores; block 4 is computed by
both endpoint cores (rowsum only) so it is never double counted.
"""

import os
import sys

import numpy as np

for _p in ("/opt/trn_rl_repo", os.path.expanduser("/root/.axon_site/_ro/trn_rl_repo")):
    if os.path.isdir(_p) and _p not in sys.path:
        sys.path.insert(0, _p)

from concourse import bass, masks, tile  # noqa: E402
from concourse.bass_utils import run_bass_kernel_spmd  # noqa: E402

mybir = bass.mybir
F32 = mybir.dt.float32
F32R = mybir.dt.float32r
BF16 = mybir.dt.bfloat16

N, D, NCORES = 8192, 64, 8
B = N // NCORES            # 1024 rows per core
MT = B // 128              # 8 row tiles of 128
TFULL = N // 128           # 64 transpose tiles
INV_T = 20.0

AP_BLOCKS = list(range(8))       # ap: all column blocks, all with colsum
SYM_BLOCKS = [0, 1, 2, 3, 4]     # aa/pp: cyclic half
SYM_COL_BLOCKS = [1, 2, 3]       # aa/pp blocks that also emit colsums


def _emit_load_normalize(nc, tc, ctx, dram_in, name):
    """DMA [N,D] -> [128,64,64] nat layout and row-normalize in place."""
    singles = ctx.enter_context(tc.tile_pool(name=f"{name}_sb", bufs=1))
    tmp = ctx.enter_context(tc.tile_pool(name=f"{name}_tmp", bufs=1))

    nat = singles.tile([128, TFULL, D], F32)
    sq = tmp.tile([128, TFULL, D], F32, tag="sqtmp")
    src = dram_in[:].rearrange("(t p) d -> p t d", p=128)
    for h in range(4):
        t0, t1 = h * (TFULL // 4), (h + 1) * (TFULL // 4)
        nc.sync.dma_start(out=nat[:, t0:t1, :], in_=src[:, t0:t1, :])
        nc.vector.tensor_mul(sq[:, t0:t1, :], nat[:, t0:t1, :], nat[:, t0:t1, :])
    ss = singles.tile([128, TFULL], F32)
    nc.vector.tensor_reduce(ss[:], sq[:], axis=mybir.AxisListType.X,
                            op=mybir.AluOpType.add)
    nrm = singles.tile([128, TFULL], F32)
    nc.scalar.activation(nrm[:], ss[:], mybir.ActivationFunctionType.Sqrt)
    inv = singles.tile([128, TFULL], F32)
    nc.vector.reciprocal(inv[:], nrm[:])
    # nat[p, t, d] *= inv[p, t]  (broadcast along d)
    inv_b = inv[:].broadcast_to([128, TFULL, D])
    nat2 = singles.tile([128, TFULL, D], F32, tag=f"{name}_nat2")
    nc.vector.tensor_mul(nat2[:], nat[:], inv_b)
    return singles, nat2


def _emit_transpose(nc, tc, singles, nat, ident, name):
    """PE-transpose normalized nat [128,64,64] -> xT [64, N].

    xT is float32r so the PSUM->SBUF copy rounds it for the full-rate
    f32r matmuls (the BIR verifier requires producers to round)."""
    xT = singles.tile([64, N], F32R, tag=f"{name}_xT")
    with tc.tile_pool(name=f"{name}_tr", bufs=2, space="PSUM") as trp:
        for q in range(TFULL // 4):
            tr = trp.tile([64, 512], F32, tag="tr")
            for s in range(4):
                t = q * 4 + s
                nc.tensor.transpose(tr[:, s * 128:(s + 1) * 128], nat[:, t, :],
                                    ident[:])
            nc.vector.tensor_copy(xT[:, q * 512:(q + 1) * 512], tr[:])
    return xT


def build_program():
    nc = bass.Bass()
    a_in = nc.declare_dram_parameter("a", [N, D], F32, isOutput=False)
    p_in = nc.declare_dram_parameter("p", [N, D], F32, isOutput=False)
    o_st_ap = nc.declare_dram_parameter("st_ap", [128, MT * 8], F32, isOutput=True)
    o_st_aa = nc.declare_dram_parameter("st_aa", [128, MT * 5], F32, isOutput=True)
    o_st_pp = nc.declare_dram_parameter("st_pp", [128, MT * 5], F32, isOutput=True)
    o_diag = nc.declare_dram_parameter("diag", [128, MT], F32, isOutput=True)
    o_cs_ap = nc.declare_dram_parameter("cs_ap", [1, 8 * B], F32, isOutput=True)
    o_cs_aa = nc.declare_dram_parameter("cs_aa", [1, 3 * B], F32, isOutput=True)
    o_cs_pp = nc.declare_dram_parameter("cs_pp", [1, 3 * B], F32, isOutput=True)

    with tile.TileContext(nc) as tc:
        import contextlib

        with contextlib.ExitStack() as ctx:
            a_sing, a_nat = _emit_load_normalize(nc, tc, ctx, a_in, "a")
            p_sing, p_nat = _emit_load_normalize(nc, tc, ctx, p_in, "p")

            res = ctx.enter_context(tc.tile_pool(name="results", bufs=1))
            st_ap = res.tile([128, MT * 8], F32)
            st_aa = res.tile([128, MT * 5], F32)
            st_pp = res.tile([128, MT * 5], F32)
            diag = res.tile([128, MT], F32)
            ones_t = res.tile([128, 128], BF16)
            nc.vector.memset(ones_t[:], 1.0)

            # diagonal cos(a_i, p_i) for own rows (block 0 of rotated input)
            dtmp = res.tile([128, MT, D], F32)
            nc.vector.tensor_mul(dtmp[:], a_nat[:, 0:MT, :], p_nat[:, 0:MT, :])
            nc.vector.tensor_reduce(diag[:], dtmp[:], axis=mybir.AxisListType.X,
                                    op=mybir.AluOpType.add)

            ident0 = res.tile([128, 128], F32)
            masks.make_identity(nc, ident0[:])
            ident = res.tile([128, 128], F32)
            nc.vector.tensor_copy(ident[:], ident0[:])
            # -100 on the diagonal: exp(20*(s-100)) == 0, so the aa/pp
            # self-terms drop out on device (no e^20 cancellation on host,
            # which f32r matmul precision cannot support)
            msk = res.tile([128, 128], F32)
            nc.vector.tensor_scalar_mul(msk[:], ident0[:], -100.0)
            aT = _emit_transpose(nc, tc, a_sing, a_nat, ident, "a")
            pT = _emit_transpose(nc, tc, p_sing, p_nat, ident, "p")

            csp = ctx.enter_context(tc.tile_pool(name="csstage", bufs=2))
            mmp = ctx.enter_context(tc.tile_pool(name="mm", bufs=2, space="PSUM"))
            colp = ctx.enter_context(tc.tile_pool(name="col", bufs=2, space="PSUM"))
            ep = ctx.enter_context(tc.tile_pool(name="etile", bufs=3))

            jobs = []
            for g, j in enumerate(AP_BLOCKS):
                jobs.append((aT, pT, j, st_ap, g, 8, o_cs_ap, j))
            for g, j in enumerate(SYM_BLOCKS):
                cj = SYM_COL_BLOCKS.index(j) if j in SYM_COL_BLOCKS else None
                jobs.append((aT, aT, j, st_aa, g, 5, o_cs_aa, cj))
            for g, j in enumerate(SYM_BLOCKS):
                cj = SYM_COL_BLOCKS.index(j) if j in SYM_COL_BLOCKS else None
                jobs.append((pT, pT, j, st_pp, g, 5, o_cs_pp, cj))

            for xT, yT, j, st, g, ng, cs, cj in jobs:
                sym_diag = xT is yT and j == 0
                col_ps = None
                if cj is not None:
                    col_ps = colp.tile([128, B], F32, tag="col")
                for m in range(MT):
                    mm_ps = mmp.tile([128, B], F32, tag="mm")
                    for c in range(2):
                        # f32r: full-rate (1 cyc/row) matmul at ~tf32 precision
                        nc.tensor.matmul(
                            mm_ps[:, c * 512:(c + 1) * 512],
                            xT[:, m * 128:(m + 1) * 128],
                            yT[:, j * B + c * 512: j * B + (c + 1) * 512],
                            start=True, stop=True,
                        )
                    if sym_diag:
                        nc.vector.tensor_add(
                            mm_ps[:, m * 128:(m + 1) * 128],
                            mm_ps[:, m * 128:(m + 1) * 128], msk[:])
                    e = ep.tile([128, B], BF16, tag="e")
                    nc.scalar.activation(
                        e[:], mm_ps[:], mybir.ActivationFunctionType.Exp,
                        scale=INV_T,
                        accum_out=st[:, m * ng + g: m * ng + g + 1],
                    )
                    if col_ps is not None:
                        for c in range(2):
                            nc.tensor.matmul(
                                col_ps[:, c * 512:(c + 1) * 512],
                                ones_t[:],
                                e[:, c * 512:(c + 1) * 512],
                                start=(m == 0), stop=(m == MT - 1),
                                skip_group_check=True,
                            )
                if col_ps is not None:
                    cstage = csp.tile([1, B], F32, tag="cs")
                    nc.scalar.activation(cstage[:], col_ps[0:1, :],
                                         mybir.ActivationFunctionType.Copy)
                    nc.sync.dma_start(out=cs[0:1, cj * B:(cj + 1) * B],
                                      in_=cstage[:])

            nc.sync.dma_start(out=o_st_ap[:], in_=st_ap[:])
            nc.sync.dma_start(out=o_st_aa[:], in_=st_aa[:])
            nc.sync.dma_start(out=o_st_pp[:], in_=st_pp[:])
            nc.sync.dma_start(out=o_diag[:], in_=diag[:])
    return nc


def combine(core_outs):
    """core_outs: list (per core) of dicts with the 7 output arrays.

    aa/pp self-terms are masked to zero on device, so no -2*exp(1/T)
    correction is needed here."""
    rs_ap = np.empty(N, np.float32)
    rs_aa = np.empty(N, np.float32)
    rs_pp = np.empty(N, np.float32)
    diag = np.empty(N, np.float32)
    cs_ap_tot = np.zeros(N, np.float64)
    aa_contrib = np.zeros(N, np.float64)
    pp_contrib = np.zeros(N, np.float64)

    for k, o in enumerate(core_outs):
        sl = slice(k * B, (k + 1) * B)
        # st[p, m*ng+g] -> local row m*128+p; sum over g
        rs_ap[sl] = o["st_ap"].reshape(128, MT, 8).sum(-1).T.reshape(B)
        rs_aa[sl] = o["st_aa"].reshape(128, MT, 5).sum(-1).T.reshape(B)
        rs_pp[sl] = o["st_pp"].reshape(128, MT, 5).sum(-1).T.reshape(B)
        diag[sl] = o["diag"].T.reshape(B)

        cs_ap_tot += np.roll(o["cs_ap"].reshape(N).astype(np.float64), k * B)
        for row, j in enumerate(SYM_COL_BLOCKS):
            v = np.zeros(N, np.float64)
            v[j * B:(j + 1) * B] = o["cs_aa"].reshape(3, B)[row]
            aa_contrib += np.roll(v, k * B)
            v = np.zeros(N, np.float64)
            v[j * B:(j + 1) * B] = o["cs_pp"].reshape(3, B)[row]
            pp_contrib += np.roll(v, k * B)

    partition = (rs_ap.astype(np.float64) + cs_ap_tot
                 + rs_aa.astype(np.float64) + aa_contrib
                 + rs_pp.astype(np.float64) + pp_contrib)
    pos_logit = INV_T * diag.astype(np.float64)
    loss = -(pos_logit - np.log(partition)).mean()
    return np.float32(loss)


def _split_waits(nc):
    """Walrus codegen allows ~1 sync wait per instruction; hoist extra
    waits onto same-engine NoOps inserted just before the instruction."""
    for fn in nc.m.functions:
        for blk in fn.blocks:
            new = []
            for inst in blk.instructions:
                si = getattr(inst, "sync_info", None)
                keep = 1
                if si is not None and si.on_wait and len(si.on_wait) > keep:
                    waits = list(si.on_wait)
                    for i, w in enumerate(waits[:-keep]):
                        nop = mybir.InstNoOp(name=f"{inst.name}-sw{i}")
                        nop.engine = inst.engine
                        nop.sync_info = mybir.SyncInfo(on_wait=[w], on_update=[])
                        new.append(nop)
                    inst.sync_info = mybir.SyncInfo(
                        on_wait=list(waits[-keep:]),
                        on_update=list(si.on_update))
                new.append(inst)
            blk.instructions = new


_NC_CACHE = None


def _get_program():
    global _NC_CACHE
    if _NC_CACHE is None:
        _NC_CACHE = build_program()
        _split_waits(_NC_CACHE)
    return _NC_CACHE


def run(anchor_embeddings, positive_embeddings, trace=False, **trace_kwargs):
    a = np.ascontiguousarray(anchor_embeddings, dtype=np.float32)
    p = np.ascontiguousarray(positive_embeddings, dtype=np.float32)
    in_maps = [
        {"a": np.roll(a, -k * B, axis=0), "p": np.roll(p, -k * B, axis=0)}
        for k in range(NCORES)
    ]
    nc = _get_program()
    res = run_bass_kernel_spmd(nc, in_maps, list(range(NCORES)), trace=trace,
                               **trace_kwargs)
    return combine(res.results), res


def kernel(anchor_embeddings, positive_embeddings):
    loss, _ = run(anchor_embeddings, positive_embeddings)
    return loss



# revision 17
# speedup vs baseline: 1.7278x; 1.1498x over previous
"""GTE contrastive loss kernel for 8 Trainium2 NeuronCores.

Math (reference): loss = -mean_i( cos(a_i,p_i)/T - log(partition_i) ),
partition_i = sum_j E_ap[i,j] + sum_j E_aa[i,j] + sum_j E_ap[j,i]
            + sum_j E_pp[j,i] - 2*exp(1/T),   E_xy = exp(cos/T).

Sharding: core k owns row block k (1024 rows).  Inputs are rotated by
-1024k rows per core so one SPMD program suffices: "my rows" are always
rows 0:1024 of the rotated input, and column block j means global block
(k+j) mod 8.  Symmetry of E_aa/E_pp lets each core compute only column
blocks 0..4: blocks 1..3 also emit column sums which cover the missing
row-sum pieces of blocks 5..7 on other cores; block 4 is computed by
both endpoint cores (rowsum only) so it is never double counted.
"""

import os
import sys

import numpy as np

for _p in ("/opt/trn_rl_repo", os.path.expanduser("/root/.axon_site/_ro/trn_rl_repo")):
    if os.path.isdir(_p) and _p not in sys.path:
        sys.path.insert(0, _p)

from concourse import bass, masks, tile  # noqa: E402
from concourse.bass_utils import run_bass_kernel_spmd  # noqa: E402

mybir = bass.mybir
F32 = mybir.dt.float32
F32R = mybir.dt.float32r
BF16 = mybir.dt.bfloat16

N, D, NCORES = 8192, 64, 8
B = N // NCORES            # 1024 rows per core
MT = B // 128              # 8 row tiles of 128
TFULL = N // 128           # 64 transpose tiles
INV_T = 20.0

AP_BLOCKS = list(range(8))       # ap: all column blocks, all with colsum
SYM_BLOCKS = [0, 1, 2, 3, 4]     # aa/pp: cyclic half
SYM_COL_BLOCKS = [1, 2, 3]       # aa/pp blocks that also emit colsums


def _emit_load_normalize(nc, tc, ctx, dram_in, name):
    """DMA [N,D] -> [128,64,64] nat layout and row-normalize in place."""
    singles = ctx.enter_context(tc.tile_pool(name=f"{name}_sb", bufs=1))
    tmp = ctx.enter_context(tc.tile_pool(name=f"{name}_tmp", bufs=1))

    nat = singles.tile([128, TFULL, D], F32)
    sq = tmp.tile([128, TFULL, D], F32, tag="sqtmp")
    src = dram_in[:].rearrange("(t p) d -> p t d", p=128)
    for h in range(4):
        t0, t1 = h * (TFULL // 4), (h + 1) * (TFULL // 4)
        nc.sync.dma_start(out=nat[:, t0:t1, :], in_=src[:, t0:t1, :])
        nc.vector.tensor_mul(sq[:, t0:t1, :], nat[:, t0:t1, :], nat[:, t0:t1, :])
    ss = singles.tile([128, TFULL], F32)
    nc.vector.tensor_reduce(ss[:], sq[:], axis=mybir.AxisListType.X,
                            op=mybir.AluOpType.add)
    nrm = singles.tile([128, TFULL], F32)
    nc.scalar.activation(nrm[:], ss[:], mybir.ActivationFunctionType.Sqrt)
    inv = singles.tile([128, TFULL], F32)
    nc.vector.reciprocal(inv[:], nrm[:])
    # nat[p, t, d] *= inv[p, t]  (broadcast along d)
    inv_b = inv[:].broadcast_to([128, TFULL, D])
    nat2 = singles.tile([128, TFULL, D], F32, tag=f"{name}_nat2")
    nc.vector.tensor_mul(nat2[:], nat[:], inv_b)
    return singles, nat2


def _emit_transpose(nc, tc, singles, nat, ident, name):
    """PE-transpose normalized nat [128,64,64] -> xT [64, N].

    xT is float32r so the PSUM->SBUF copy rounds it for the full-rate
    f32r matmuls (the BIR verifier requires producers to round)."""
    xT = singles.tile([64, N], F32R, tag=f"{name}_xT")
    with tc.tile_pool(name=f"{name}_tr", bufs=2, space="PSUM") as trp:
        for q in range(TFULL // 4):
            tr = trp.tile([64, 512], F32, tag="tr")
            for s in range(4):
                t = q * 4 + s
                nc.tensor.transpose(tr[:, s * 128:(s + 1) * 128], nat[:, t, :],
                                    ident[:])
            nc.vector.tensor_copy(xT[:, q * 512:(q + 1) * 512], tr[:])
    return xT


def build_program():
    nc = bass.Bass()
    a_in = nc.declare_dram_parameter("a", [N, D], F32, isOutput=False)
    p_in = nc.declare_dram_parameter("p", [N, D], F32, isOutput=False)
    o_st_ap = nc.declare_dram_parameter("st_ap", [128, MT * 8], F32, isOutput=True)
    o_st_aa = nc.declare_dram_parameter("st_aa", [128, MT * 5], F32, isOutput=True)
    o_st_pp = nc.declare_dram_parameter("st_pp", [128, MT * 5], F32, isOutput=True)
    o_diag = nc.declare_dram_parameter("diag", [128, MT], F32, isOutput=True)
    o_cs_ap = nc.declare_dram_parameter("cs_ap", [1, 8 * B], F32, isOutput=True)
    o_cs_aa = nc.declare_dram_parameter("cs_aa", [1, 3 * B], F32, isOutput=True)
    o_cs_pp = nc.declare_dram_parameter("cs_pp", [1, 3 * B], F32, isOutput=True)

    with tile.TileContext(nc) as tc:
        import contextlib

        with contextlib.ExitStack() as ctx:
            a_sing, a_nat = _emit_load_normalize(nc, tc, ctx, a_in, "a")
            p_sing, p_nat = _emit_load_normalize(nc, tc, ctx, p_in, "p")

            res = ctx.enter_context(tc.tile_pool(name="results", bufs=1))
            st_ap = res.tile([128, MT * 8], F32)
            st_aa = res.tile([128, MT * 5], F32)
            st_pp = res.tile([128, MT * 5], F32)
            diag = res.tile([128, MT], F32)
            ones_t = res.tile([128, 128], BF16)
            nc.vector.memset(ones_t[:], 1.0)

            # diagonal cos(a_i, p_i) for own rows (block 0 of rotated input)
            dtmp = res.tile([128, MT, D], F32)
            nc.vector.tensor_mul(dtmp[:], a_nat[:, 0:MT, :], p_nat[:, 0:MT, :])
            nc.vector.tensor_reduce(diag[:], dtmp[:], axis=mybir.AxisListType.X,
                                    op=mybir.AluOpType.add)

            ident0 = res.tile([128, 128], F32)
            masks.make_identity(nc, ident0[:])
            ident = res.tile([128, 128], F32)
            nc.vector.tensor_copy(ident[:], ident0[:])
            # -100 on the diagonal: exp(20*(s-100)) == 0, so the aa/pp
            # self-terms drop out on device (no e^20 cancellation on host,
            # which f32r matmul precision cannot support)
            msk = res.tile([128, 128], F32)
            nc.vector.tensor_scalar_mul(msk[:], ident0[:], -100.0)
            aT = _emit_transpose(nc, tc, a_sing, a_nat, ident, "a")
            pT = _emit_transpose(nc, tc, p_sing, p_nat, ident, "p")

            csp = ctx.enter_context(tc.tile_pool(name="csstage", bufs=2))
            mmp = ctx.enter_context(tc.tile_pool(name="mm", bufs=3, space="PSUM"))
            colp = ctx.enter_context(tc.tile_pool(name="col", bufs=1, space="PSUM"))
            ep = ctx.enter_context(tc.tile_pool(name="etile", bufs=3))

            jobs = []
            for g, j in enumerate(AP_BLOCKS):
                jobs.append((aT, pT, j, st_ap, g, 8, o_cs_ap, j))
            for g, j in enumerate(SYM_BLOCKS):
                cj = SYM_COL_BLOCKS.index(j) if j in SYM_COL_BLOCKS else None
                jobs.append((aT, aT, j, st_aa, g, 5, o_cs_aa, cj))
            for g, j in enumerate(SYM_BLOCKS):
                cj = SYM_COL_BLOCKS.index(j) if j in SYM_COL_BLOCKS else None
                jobs.append((pT, pT, j, st_pp, g, 5, o_cs_pp, cj))

            # Colsum matmuls are deferred by one tile so the (in-order) PE
            # stream never waits on the exp that produces e: the colsum of
            # tile t issues after the sim matmuls of tile t+1.
            pending = []

            def _flush_pending():
                while pending:
                    pending.pop(0)()

            def _make_col(col_ps, e, m, cs, cj):
                def emit():
                    for c in range(2):
                        nc.tensor.matmul(
                            col_ps[:, c * 512:(c + 1) * 512],
                            ones_t[:],
                            e[:, c * 512:(c + 1) * 512],
                            start=(m == 0), stop=(m == MT - 1),
                            skip_group_check=True,
                        )
                    if m == MT - 1:
                        # stage via DVE (not scalar: keep exp unblocked)
                        cstage = csp.tile([1, B], F32, tag="cs")
                        nc.vector.tensor_copy(cstage[:], col_ps[0:1, :])
                        nc.sync.dma_start(out=cs[0:1, cj * B:(cj + 1) * B],
                                          in_=cstage[:])
                return emit

            for xT, yT, j, st, g, ng, cs, cj in jobs:
                sym_diag = xT is yT and j == 0
                col_ps = None
                if cj is not None:
                    col_ps = colp.tile([128, B], F32, tag="col")
                for m in range(MT):
                    mm_ps = mmp.tile([128, B], F32, tag="mm")
                    for c in range(2):
                        # f32r: full-rate (1 cyc/row) matmul at ~tf32 precision
                        nc.tensor.matmul(
                            mm_ps[:, c * 512:(c + 1) * 512],
                            xT[:, m * 128:(m + 1) * 128],
                            yT[:, j * B + c * 512: j * B + (c + 1) * 512],
                            start=True, stop=True,
                        )
                    _flush_pending()
                    if sym_diag:
                        nc.vector.tensor_add(
                            mm_ps[:, m * 128:(m + 1) * 128],
                            mm_ps[:, m * 128:(m + 1) * 128], msk[:])
                    e = ep.tile([128, B], BF16, tag="e")
                    nc.scalar.activation(
                        e[:], mm_ps[:], mybir.ActivationFunctionType.Exp,
                        scale=INV_T,
                        accum_out=st[:, m * ng + g: m * ng + g + 1],
                    )
                    if col_ps is not None:
                        pending.append(_make_col(col_ps, e, m, cs, cj))
            _flush_pending()

            nc.sync.dma_start(out=o_st_ap[:], in_=st_ap[:])
            nc.sync.dma_start(out=o_st_aa[:], in_=st_aa[:])
            nc.sync.dma_start(out=o_st_pp[:], in_=st_pp[:])
            nc.sync.dma_start(out=o_diag[:], in_=diag[:])
    return nc


def combine(core_outs):
    """core_outs: list (per core) of dicts with the 7 output arrays.

    aa/pp self-terms are masked to zero on device, so no -2*exp(1/T)
    correction is needed here."""
    rs_ap = np.empty(N, np.float32)
    rs_aa = np.empty(N, np.float32)
    rs_pp = np.empty(N, np.float32)
    diag = np.empty(N, np.float32)
    cs_ap_tot = np.zeros(N, np.float64)
    aa_contrib = np.zeros(N, np.float64)
    pp_contrib = np.zeros(N, np.float64)

    for k, o in enumerate(core_outs):
        sl = slice(k * B, (k + 1) * B)
        # st[p, m*ng+g] -> local row m*128+p; sum over g
        rs_ap[sl] = o["st_ap"].reshape(128, MT, 8).sum(-1).T.reshape(B)
        rs_aa[sl] = o["st_aa"].reshape(128, MT, 5).sum(-1).T.reshape(B)
        rs_pp[sl] = o["st_pp"].reshape(128, MT, 5).sum(-1).T.reshape(B)
        diag[sl] = o["diag"].T.reshape(B)

        cs_ap_tot += np.roll(o["cs_ap"].reshape(N).astype(np.float64), k * B)
        for row, j in enumerate(SYM_COL_BLOCKS):
            v = np.zeros(N, np.float64)
            v[j * B:(j + 1) * B] = o["cs_aa"].reshape(3, B)[row]
            aa_contrib += np.roll(v, k * B)
            v = np.zeros(N, np.float64)
            v[j * B:(j + 1) * B] = o["cs_pp"].reshape(3, B)[row]
            pp_contrib += np.roll(v, k * B)

    partition = (rs_ap.astype(np.float64) + cs_ap_tot
                 + rs_aa.astype(np.float64) + aa_contrib
                 + rs_pp.astype(np.float64) + pp_contrib)
    pos_logit = INV_T * diag.astype(np.float64)
    loss = -(pos_logit - np.log(partition)).mean()
    return np.float32(loss)


def _split_waits(nc):
    """Walrus codegen allows ~1 sync wait per instruction; hoist extra
    waits onto same-engine NoOps inserted just before the instruction."""
    for fn in nc.m.functions:
        for blk in fn.blocks:
            new = []
            for inst in blk.instructions:
                si = getattr(inst, "sync_info", None)
                keep = 1
                if si is not None and si.on_wait and len(si.on_wait) > keep:
                    waits = list(si.on_wait)
                    for i, w in enumerate(waits[:-keep]):
                        nop = mybir.InstNoOp(name=f"{inst.name}-sw{i}")
                        nop.engine = inst.engine
                        nop.sync_info = mybir.SyncInfo(on_wait=[w], on_update=[])
                        new.append(nop)
                    inst.sync_info = mybir.SyncInfo(
                        on_wait=list(waits[-keep:]),
                        on_update=list(si.on_update))
                new.append(inst)
            blk.instructions = new


_NC_CACHE = None


def _get_program():
    global _NC_CACHE
    if _NC_CACHE is None:
        _NC_CACHE = build_program()
        _split_waits(_NC_CACHE)
    return _NC_CACHE


def run(anchor_embeddings, positive_embeddings, trace=False, **trace_kwargs):
    a = np.ascontiguousarray(anchor_embeddings, dtype=np.float32)
    p = np.ascontiguousarray(positive_embeddings, dtype=np.float32)
    in_maps = [
        {"a": np.roll(a, -k * B, axis=0), "p": np.roll(p, -k * B, axis=0)}
        for k in range(NCORES)
    ]
    nc = _get_program()
    res = run_bass_kernel_spmd(nc, in_maps, list(range(NCORES)), trace=trace,
                               **trace_kwargs)
    return combine(res.results), res


def kernel(anchor_embeddings, positive_embeddings):
    loss, _ = run(anchor_embeddings, positive_embeddings)
    return loss



# revision 20
# speedup vs baseline: 2.0339x; 1.1772x over previous
"""GTE contrastive loss kernel for 8 Trainium2 NeuronCores.

Math (reference): loss = -mean_i( cos(a_i,p_i)/T - log(partition_i) ),
partition_i = sum_j E_ap[i,j] + sum_j E_aa[i,j] + sum_j E_ap[j,i]
            + sum_j E_pp[j,i] - 2*exp(1/T),   E_xy = exp(cos/T).

Sharding: core k owns row block k (1024 rows).  Inputs are rotated by
-1024k rows per core so one SPMD program suffices: "my rows" are always
rows 0:1024 of the rotated input, and column block j means global block
(k+j) mod 8.  Symmetry of E_aa/E_pp lets each core compute only column
blocks 0..4: blocks 1..3 also emit column sums which cover the missing
row-sum pieces of blocks 5..7 on other cores; block 4 is computed by
both endpoint cores (rowsum only) so it is never double counted.
"""

import os
import sys

import numpy as np

for _p in ("/opt/trn_rl_repo", os.path.expanduser("/root/.axon_site/_ro/trn_rl_repo")):
    if os.path.isdir(_p) and _p not in sys.path:
        sys.path.insert(0, _p)

from concourse import bass, masks, tile  # noqa: E402
from concourse.bass_utils import run_bass_kernel_spmd  # noqa: E402

mybir = bass.mybir
F32 = mybir.dt.float32
F32R = mybir.dt.float32r
BF16 = mybir.dt.bfloat16

N, D, NCORES = 8192, 64, 8
B = N // NCORES            # 1024 rows per core
MT = B // 128              # 8 row tiles of 128
TFULL = N // 128           # 64 transpose tiles
INV_T = 20.0

AP_BLOCKS = list(range(8))       # ap: all column blocks, all with colsum
SYM_BLOCKS = [0, 1, 2, 3, 4]     # aa/pp: cyclic half
SYM_COL_BLOCKS = [1, 2, 3]       # aa/pp blocks that also emit colsums


def _emit_load_normalize(nc, tc, ctx, dram_in, name):
    """DMA [N,D] -> [128,64,64] nat layout and row-normalize in place."""
    singles = ctx.enter_context(tc.tile_pool(name=f"{name}_sb", bufs=1))
    tmp = ctx.enter_context(tc.tile_pool(name=f"{name}_tmp", bufs=1))

    nat = singles.tile([128, TFULL, D], F32)
    sq = tmp.tile([128, TFULL, D], F32, tag="sqtmp")
    src = dram_in[:].rearrange("(t p) d -> p t d", p=128)
    for h in range(4):
        t0, t1 = h * (TFULL // 4), (h + 1) * (TFULL // 4)
        nc.sync.dma_start(out=nat[:, t0:t1, :], in_=src[:, t0:t1, :])
        nc.vector.tensor_mul(sq[:, t0:t1, :], nat[:, t0:t1, :], nat[:, t0:t1, :])
    ss = singles.tile([128, TFULL], F32)
    nc.vector.tensor_reduce(ss[:], sq[:], axis=mybir.AxisListType.X,
                            op=mybir.AluOpType.add)
    nrm = singles.tile([128, TFULL], F32)
    nc.scalar.activation(nrm[:], ss[:], mybir.ActivationFunctionType.Sqrt)
    inv = singles.tile([128, TFULL], F32)
    nc.vector.reciprocal(inv[:], nrm[:])
    # nat[p, t, d] *= inv[p, t]  (broadcast along d)
    inv_b = inv[:].broadcast_to([128, TFULL, D])
    nat2 = singles.tile([128, TFULL, D], F32, tag=f"{name}_nat2")
    nc.vector.tensor_mul(nat2[:], nat[:], inv_b)
    return singles, nat2


def _emit_transpose(nc, tc, singles, nat, ident, name):
    """PE-transpose normalized nat [128,64,64] -> xT [64, N].

    xT is float32r so the PSUM->SBUF copy rounds it for the full-rate
    f32r matmuls (the BIR verifier requires producers to round)."""
    xT = singles.tile([64, N], F32R, tag=f"{name}_xT")
    with tc.tile_pool(name=f"{name}_tr", bufs=2, space="PSUM") as trp:
        for q in range(TFULL // 4):
            tr = trp.tile([64, 512], F32, tag="tr")
            for s in range(4):
                t = q * 4 + s
                nc.tensor.transpose(tr[:, s * 128:(s + 1) * 128], nat[:, t, :],
                                    ident[:])
            nc.vector.tensor_copy(xT[:, q * 512:(q + 1) * 512], tr[:])
    return xT


def build_program():
    nc = bass.Bass()
    a_in = nc.declare_dram_parameter("a", [N, D], F32, isOutput=False)
    p_in = nc.declare_dram_parameter("p", [N, D], F32, isOutput=False)
    o_st_ap = nc.declare_dram_parameter("st_ap", [128, MT * 8], F32, isOutput=True)
    o_st_aa = nc.declare_dram_parameter("st_aa", [128, MT * 5], F32, isOutput=True)
    o_st_pp = nc.declare_dram_parameter("st_pp", [128, MT * 5], F32, isOutput=True)
    o_diag = nc.declare_dram_parameter("diag", [128, MT], F32, isOutput=True)
    o_cs_ap = nc.declare_dram_parameter("cs_ap", [1, 8 * B], F32, isOutput=True)
    o_cs_aa = nc.declare_dram_parameter("cs_aa", [1, 3 * B], F32, isOutput=True)
    o_cs_pp = nc.declare_dram_parameter("cs_pp", [1, 3 * B], F32, isOutput=True)

    with tile.TileContext(nc) as tc:
        import contextlib

        with contextlib.ExitStack() as ctx:
            a_sing, a_nat = _emit_load_normalize(nc, tc, ctx, a_in, "a")
            p_sing, p_nat = _emit_load_normalize(nc, tc, ctx, p_in, "p")

            res = ctx.enter_context(tc.tile_pool(name="results", bufs=1))
            st_ap = res.tile([128, MT * 8], F32)
            st_aa = res.tile([128, MT * 5], F32)
            st_pp = res.tile([128, MT * 5], F32)
            diag = res.tile([128, MT], F32)
            ones_t = res.tile([128, 128], BF16)
            nc.vector.memset(ones_t[:], 1.0)

            # diagonal cos(a_i, p_i) for own rows (block 0 of rotated input)
            dtmp = res.tile([128, MT, D], F32)
            nc.vector.tensor_mul(dtmp[:], a_nat[:, 0:MT, :], p_nat[:, 0:MT, :])
            nc.vector.tensor_reduce(diag[:], dtmp[:], axis=mybir.AxisListType.X,
                                    op=mybir.AluOpType.add)

            ident0 = res.tile([128, 128], F32)
            masks.make_identity(nc, ident0[:])
            ident = res.tile([128, 128], F32)
            nc.vector.tensor_copy(ident[:], ident0[:])
            # -100 on the diagonal: exp(20*(s-100)) == 0, so the aa/pp
            # self-terms drop out on device (no e^20 cancellation on host,
            # which f32r matmul precision cannot support)
            msk = res.tile([128, 128], F32)
            nc.vector.tensor_scalar_mul(msk[:], ident0[:], -100.0)
            aT = _emit_transpose(nc, tc, a_sing, a_nat, ident, "a")

            csp = ctx.enter_context(tc.tile_pool(name="csstage", bufs=2))
            mmp = ctx.enter_context(tc.tile_pool(name="mm", bufs=2, space="PSUM"))
            colp = ctx.enter_context(tc.tile_pool(name="col", bufs=1, space="PSUM"))
            ep = ctx.enter_context(tc.tile_pool(name="etile", bufs=3))
            sump = ctx.enter_context(tc.tile_pool(name="esum", bufs=2))

            # aa jobs run first (they only need aT); pT transposes are
            # emitted after two aa jobs so the PE transpose burst overlaps
            # with scalar exp work instead of extending the prologue.
            jobs = []
            for g, j in enumerate(SYM_BLOCKS):
                cj = SYM_COL_BLOCKS.index(j) if j in SYM_COL_BLOCKS else None
                jobs.append(("aa", j, st_aa, g, 5, o_cs_aa, cj))
            for g, j in enumerate(AP_BLOCKS):
                jobs.append(("ap", j, st_ap, g, 8, o_cs_ap, j))
            for g, j in enumerate(SYM_BLOCKS):
                cj = SYM_COL_BLOCKS.index(j) if j in SYM_COL_BLOCKS else None
                jobs.append(("pp", j, st_pp, g, 5, o_cs_pp, cj))

            # Colsum work is deferred so the (in-order) PE stream never
            # waits on exp/DVE: a job's two colsum matmuls (on the
            # DVE-accumulated e-sum) issue only after the next job has
            # started its sim matmuls.
            pending = []

            def _flush_pending():
                while pending:
                    pending.pop(0)()

            def _make_col(esum, cs, cj):
                def emit():
                    col_ps = colp.tile([128, B], F32, tag="col")
                    for c in range(2):
                        nc.tensor.matmul(
                            col_ps[:, c * 512:(c + 1) * 512],
                            ones_t[:],
                            esum[:, c * 512:(c + 1) * 512],
                            start=True, stop=True,
                        )
                    # stage via DVE (not scalar: keep exp unblocked)
                    cstage = csp.tile([1, B], F32, tag="cs")
                    nc.vector.tensor_copy(cstage[:], col_ps[0:1, :])
                    nc.sync.dma_start(out=cs[0:1, cj * B:(cj + 1) * B],
                                      in_=cstage[:])
                return emit

            pT = None
            for jn, (kind, j, st, g, ng, cs, cj) in enumerate(jobs):
                if (jn == 2 or kind != "aa") and pT is None:
                    pT = _emit_transpose(nc, tc, p_sing, p_nat, ident, "p")
                xT = aT if kind in ("aa", "ap") else pT
                yT = pT if kind in ("ap", "pp") else aT
                sym_diag = kind != "ap" and j == 0
                esum = None
                e_first = None
                if cj is not None:
                    esum = sump.tile([128, B], BF16, tag="esum")
                for m in range(MT):
                    mm_ps = mmp.tile([128, B], F32, tag="mm")
                    for c in range(2):
                        # f32r: full-rate (1 cyc/row) matmul at ~tf32 precision
                        nc.tensor.matmul(
                            mm_ps[:, c * 512:(c + 1) * 512],
                            xT[:, m * 128:(m + 1) * 128],
                            yT[:, j * B + c * 512: j * B + (c + 1) * 512],
                            start=True, stop=True,
                        )
                    if m == 1:
                        _flush_pending()
                    if sym_diag:
                        nc.vector.tensor_add(
                            mm_ps[:, m * 128:(m + 1) * 128],
                            mm_ps[:, m * 128:(m + 1) * 128], msk[:])
                    e = ep.tile([128, B], BF16, tag="e")
                    nc.scalar.activation(
                        e[:], mm_ps[:], mybir.ActivationFunctionType.Exp,
                        scale=INV_T,
                        accum_out=st[:, m * ng + g: m * ng + g + 1],
                    )
                    if esum is not None:
                        # running bf16 e-sum on DVE; replaces the per-tile
                        # PE colsum matmuls (8x fewer PE rows)
                        if m == 0:
                            e_first = e
                        elif m == 1:
                            nc.vector.tensor_add(esum[:], e_first[:], e[:])
                        else:
                            nc.vector.tensor_add(esum[:], esum[:], e[:])
                if esum is not None:
                    pending.append(_make_col(esum, cs, cj))
            _flush_pending()

            nc.sync.dma_start(out=o_st_ap[:], in_=st_ap[:])
            nc.sync.dma_start(out=o_st_aa[:], in_=st_aa[:])
            nc.sync.dma_start(out=o_st_pp[:], in_=st_pp[:])
            nc.sync.dma_start(out=o_diag[:], in_=diag[:])
    return nc


def combine(core_outs):
    """core_outs: list (per core) of dicts with the 7 output arrays.

    aa/pp self-terms are masked to zero on device, so no -2*exp(1/T)
    correction is needed here."""
    rs_ap = np.empty(N, np.float32)
    rs_aa = np.empty(N, np.float32)
    rs_pp = np.empty(N, np.float32)
    diag = np.empty(N, np.float32)
    cs_ap_tot = np.zeros(N, np.float64)
    aa_contrib = np.zeros(N, np.float64)
    pp_contrib = np.zeros(N, np.float64)

    for k, o in enumerate(core_outs):
        sl = slice(k * B, (k + 1) * B)
        # st[p, m*ng+g] -> local row m*128+p; sum over g
        rs_ap[sl] = o["st_ap"].reshape(128, MT, 8).sum(-1).T.reshape(B)
        rs_aa[sl] = o["st_aa"].reshape(128, MT, 5).sum(-1).T.reshape(B)
        rs_pp[sl] = o["st_pp"].reshape(128, MT, 5).sum(-1).T.reshape(B)
        diag[sl] = o["diag"].T.reshape(B)

        cs_ap_tot += np.roll(o["cs_ap"].reshape(N).astype(np.float64), k * B)
        for row, j in enumerate(SYM_COL_BLOCKS):
            v = np.zeros(N, np.float64)
            v[j * B:(j + 1) * B] = o["cs_aa"].reshape(3, B)[row]
            aa_contrib += np.roll(v, k * B)
            v = np.zeros(N, np.float64)
            v[j * B:(j + 1) * B] = o["cs_pp"].reshape(3, B)[row]
            pp_contrib += np.roll(v, k * B)

    partition = (rs_ap.astype(np.float64) + cs_ap_tot
                 + rs_aa.astype(np.float64) + aa_contrib
                 + rs_pp.astype(np.float64) + pp_contrib)
    pos_logit = INV_T * diag.astype(np.float64)
    loss = -(pos_logit - np.log(partition)).mean()
    return np.float32(loss)


def _split_waits(nc):
    """Walrus codegen allows ~1 sync wait per instruction; hoist extra
    waits onto same-engine NoOps inserted just before the instruction."""
    for fn in nc.m.functions:
        for blk in fn.blocks:
            new = []
            for inst in blk.instructions:
                si = getattr(inst, "sync_info", None)
                keep = 1
                if si is not None and si.on_wait and len(si.on_wait) > keep:
                    waits = list(si.on_wait)
                    for i, w in enumerate(waits[:-keep]):
                        nop = mybir.InstNoOp(name=f"{inst.name}-sw{i}")
                        nop.engine = inst.engine
                        nop.sync_info = mybir.SyncInfo(on_wait=[w], on_update=[])
                        new.append(nop)
                    inst.sync_info = mybir.SyncInfo(
                        on_wait=list(waits[-keep:]),
                        on_update=list(si.on_update))
                new.append(inst)
            blk.instructions = new


_NC_CACHE = None


def _get_program():
    global _NC_CACHE
    if _NC_CACHE is None:
        _NC_CACHE = build_program()
        _split_waits(_NC_CACHE)
    return _NC_CACHE


def run(anchor_embeddings, positive_embeddings, trace=False, **trace_kwargs):
    a = np.ascontiguousarray(anchor_embeddings, dtype=np.float32)
    p = np.ascontiguousarray(positive_embeddings, dtype=np.float32)
    in_maps = [
        {"a": np.roll(a, -k * B, axis=0), "p": np.roll(p, -k * B, axis=0)}
        for k in range(NCORES)
    ]
    nc = _get_program()
    res = run_bass_kernel_spmd(nc, in_maps, list(range(NCORES)), trace=trace,
                               **trace_kwargs)
    return combine(res.results), res


def kernel(anchor_embeddings, positive_embeddings):
    loss, _ = run(anchor_embeddings, positive_embeddings)
    return loss



# revision 24
# speedup vs baseline: 2.1741x; 1.0689x over previous
"""GTE contrastive loss kernel for 8 Trainium2 NeuronCores.

Math (reference): loss = -mean_i( cos(a_i,p_i)/T - log(partition_i) ),
partition_i = sum_j E_ap[i,j] + sum_j E_aa[i,j] + sum_j E_ap[j,i]
            + sum_j E_pp[j,i] - 2*exp(1/T),   E_xy = exp(cos/T).

Sharding: core k owns row block k (1024 rows).  Inputs are rotated by
-1024k rows per core so one SPMD program suffices: "my rows" are always
rows 0:1024 of the rotated input, and column block j means global block
(k+j) mod 8.  Symmetry of E_aa/E_pp lets each core compute only column
blocks 0..4: blocks 1..3 also emit column sums which cover the missing
row-sum pieces of blocks 5..7 on other cores; block 4 is computed by
both endpoint cores (rowsum only) so it is never double counted.
"""

import os
import sys

import numpy as np

for _p in ("/opt/trn_rl_repo", os.path.expanduser("/root/.axon_site/_ro/trn_rl_repo")):
    if os.path.isdir(_p) and _p not in sys.path:
        sys.path.insert(0, _p)

from concourse import bass, masks, tile  # noqa: E402
from concourse.bass_utils import run_bass_kernel_spmd  # noqa: E402

mybir = bass.mybir
F32 = mybir.dt.float32
F32R = mybir.dt.float32r
BF16 = mybir.dt.bfloat16

N, D, NCORES = 8192, 64, 8
B = N // NCORES            # 1024 rows per core
MT = B // 128              # 8 row tiles of 128
TFULL = N // 128           # 64 transpose tiles
INV_T = 20.0

AP_BLOCKS = list(range(8))       # ap: all column blocks, all with colsum
SYM_BLOCKS = [0, 1, 2, 3, 4]     # aa/pp: cyclic half
SYM_COL_BLOCKS = [1, 2, 3]       # aa/pp blocks that also emit colsums


def _emit_load_normalize(nc, tc, ctx, dram_in, name):
    """DMA [N,D] -> [128,64,64] nat layout and row-normalize in place."""
    singles = ctx.enter_context(tc.tile_pool(name=f"{name}_sb", bufs=1))
    tmp = ctx.enter_context(tc.tile_pool(name=f"{name}_tmp", bufs=1))

    nat = singles.tile([128, TFULL, D], F32)
    sq = tmp.tile([128, TFULL, D], F32, tag="sqtmp")
    src = dram_in[:].rearrange("(t p) d -> p t d", p=128)
    for h in range(4):
        t0, t1 = h * (TFULL // 4), (h + 1) * (TFULL // 4)
        nc.sync.dma_start(out=nat[:, t0:t1, :], in_=src[:, t0:t1, :])
        nc.vector.tensor_mul(sq[:, t0:t1, :], nat[:, t0:t1, :], nat[:, t0:t1, :])
    ss = singles.tile([128, TFULL], F32)
    nc.vector.tensor_reduce(ss[:], sq[:], axis=mybir.AxisListType.X,
                            op=mybir.AluOpType.add)
    nrm = singles.tile([128, TFULL], F32)
    nc.scalar.activation(nrm[:], ss[:], mybir.ActivationFunctionType.Sqrt)
    inv = singles.tile([128, TFULL], F32)
    nc.vector.reciprocal(inv[:], nrm[:])
    # nat[p, t, d] *= inv[p, t]  (broadcast along d); bf16 out feeds the
    # bf16 transposes + matmuls
    inv_b = inv[:].broadcast_to([128, TFULL, D])
    nat2 = singles.tile([128, TFULL, D], BF16, tag=f"{name}_nat2")
    nc.vector.tensor_mul(nat2[:], nat[:], inv_b)
    return singles, nat2


def _emit_transpose(nc, tc, singles, nat, ident, name):
    """PE-transpose normalized bf16 nat [128,64,64] -> xT [64, N]."""
    xT = singles.tile([64, N], BF16, tag=f"{name}_xT")
    with tc.tile_pool(name=f"{name}_tr", bufs=2, space="PSUM") as trp:
        for q in range(TFULL // 4):
            tr = trp.tile([64, 512], BF16, tag="tr")
            for s in range(4):
                t = q * 4 + s
                nc.tensor.transpose(tr[:, s * 128:(s + 1) * 128], nat[:, t, :],
                                    ident[:])
            nc.vector.tensor_copy(xT[:, q * 512:(q + 1) * 512], tr[:])
    return xT


def build_program():
    nc = bass.Bass()
    a_in = nc.declare_dram_parameter("a", [N, D], F32, isOutput=False)
    p_in = nc.declare_dram_parameter("p", [N, D], F32, isOutput=False)
    o_st_ap = nc.declare_dram_parameter("st_ap", [128, MT * 8], F32, isOutput=True)
    o_st_aa = nc.declare_dram_parameter("st_aa", [128, MT * 5], F32, isOutput=True)
    o_st_pp = nc.declare_dram_parameter("st_pp", [128, MT * 5], F32, isOutput=True)
    o_diag = nc.declare_dram_parameter("diag", [128, MT], F32, isOutput=True)
    o_cs_ap = nc.declare_dram_parameter("cs_ap", [1, 8 * B], F32, isOutput=True)
    o_cs_aa = nc.declare_dram_parameter("cs_aa", [1, 3 * B], F32, isOutput=True)
    o_cs_pp = nc.declare_dram_parameter("cs_pp", [1, 3 * B], F32, isOutput=True)

    with tile.TileContext(nc) as tc:
        import contextlib

        with contextlib.ExitStack() as ctx:
            a_sing, a_nat = _emit_load_normalize(nc, tc, ctx, a_in, "a")
            p_sing, p_nat = _emit_load_normalize(nc, tc, ctx, p_in, "p")

            res = ctx.enter_context(tc.tile_pool(name="results", bufs=1))
            st_ap = res.tile([128, MT * 8], F32)
            st_aa = res.tile([128, MT * 5], F32)
            st_pp = res.tile([128, MT * 5], F32)
            diag = res.tile([128, MT], F32)
            ones_t = res.tile([128, 128], BF16)
            nc.vector.memset(ones_t[:], 1.0)

            # diagonal cos(a_i, p_i) for own rows (block 0 of rotated input)
            dtmp = res.tile([128, MT, D], F32)
            nc.vector.tensor_mul(dtmp[:], a_nat[:, 0:MT, :], p_nat[:, 0:MT, :])
            nc.vector.tensor_reduce(diag[:], dtmp[:], axis=mybir.AxisListType.X,
                                    op=mybir.AluOpType.add)

            ident0 = res.tile([128, 128], F32)
            masks.make_identity(nc, ident0[:])
            ident = res.tile([128, 128], BF16)
            nc.vector.tensor_copy(ident[:], ident0[:])
            # -100 on the diagonal: exp(20*(s-100)) == 0, so the aa/pp
            # self-terms drop out on device (no e^20 cancellation on host,
            # which f32r matmul precision cannot support)
            msk = res.tile([128, 128], F32)
            nc.vector.tensor_scalar_mul(msk[:], ident0[:], -100.0)
            aT = _emit_transpose(nc, tc, a_sing, a_nat, ident, "a")

            csp = ctx.enter_context(tc.tile_pool(name="csstage", bufs=2))
            mmp = ctx.enter_context(tc.tile_pool(name="mm", bufs=2, space="PSUM"))
            colp = ctx.enter_context(tc.tile_pool(name="col", bufs=1, space="PSUM"))
            ep = ctx.enter_context(tc.tile_pool(name="etile", bufs=3))
            sump = ctx.enter_context(tc.tile_pool(name="esum", bufs=2))

            # aa jobs run first (they only need aT); pT transposes are
            # emitted after two aa jobs so the PE transpose burst overlaps
            # with scalar exp work instead of extending the prologue.
            jobs = []
            for g, j in enumerate(SYM_BLOCKS):
                cj = SYM_COL_BLOCKS.index(j) if j in SYM_COL_BLOCKS else None
                jobs.append(("aa", j, st_aa, g, 5, o_cs_aa, cj))
            for g, j in enumerate(AP_BLOCKS):
                jobs.append(("ap", j, st_ap, g, 8, o_cs_ap, j))
            for g, j in enumerate(SYM_BLOCKS):
                cj = SYM_COL_BLOCKS.index(j) if j in SYM_COL_BLOCKS else None
                jobs.append(("pp", j, st_pp, g, 5, o_cs_pp, cj))

            # Colsum work is deferred so the (in-order) PE stream never
            # waits on exp/DVE: a job's two colsum matmuls (on the
            # DVE-accumulated e-sum) issue only after the next job has
            # started its sim matmuls.
            pending = []

            def _flush_pending():
                while pending:
                    pending.pop(0)()

            def _make_col(esum, cs, cj):
                def emit():
                    col_ps = colp.tile([128, B], F32, tag="col")
                    for c in range(2):
                        nc.tensor.matmul(
                            col_ps[:, c * 512:(c + 1) * 512],
                            ones_t[:],
                            esum[:, c * 512:(c + 1) * 512],
                            start=True, stop=True,
                        )
                    # stage via DVE (not scalar: keep exp unblocked)
                    cstage = csp.tile([1, B], F32, tag="cs")
                    nc.vector.tensor_copy(cstage[:], col_ps[0:1, :])
                    nc.sync.dma_start(out=cs[0:1, cj * B:(cj + 1) * B],
                                      in_=cstage[:])
                return emit

            pT = None
            for jn, (kind, j, st, g, ng, cs, cj) in enumerate(jobs):
                if (jn == 2 or kind != "aa") and pT is None:
                    pT = _emit_transpose(nc, tc, p_sing, p_nat, ident, "p")
                xT = aT if kind in ("aa", "ap") else pT
                yT = pT if kind in ("ap", "pp") else aT
                sym_diag = kind != "ap" and j == 0
                esum = None
                e_first = None
                if cj is not None:
                    esum = sump.tile([128, B], BF16, tag="esum")
                for m in range(MT):
                    mm_ps = mmp.tile([128, B], F32, tag="mm")
                    for c in range(2):
                        # bf16 operands: halves moving-operand bytes/column
                        nc.tensor.matmul(
                            mm_ps[:, c * 512:(c + 1) * 512],
                            xT[:, m * 128:(m + 1) * 128],
                            yT[:, j * B + c * 512: j * B + (c + 1) * 512],
                            start=True, stop=True,
                        )
                    if m == 1:
                        _flush_pending()
                    if sym_diag:
                        nc.vector.tensor_add(
                            mm_ps[:, m * 128:(m + 1) * 128],
                            mm_ps[:, m * 128:(m + 1) * 128], msk[:])
                    e = ep.tile([128, B], BF16, tag="e")
                    nc.scalar.activation(
                        e[:], mm_ps[:], mybir.ActivationFunctionType.Exp,
                        scale=INV_T,
                        accum_out=st[:, m * ng + g: m * ng + g + 1],
                    )
                    if esum is not None:
                        # running bf16 e-sum on DVE; replaces the per-tile
                        # PE colsum matmuls (8x fewer PE rows)
                        if m == 0:
                            e_first = e
                        elif m == 1:
                            nc.vector.tensor_add(esum[:], e_first[:], e[:])
                        else:
                            nc.vector.tensor_add(esum[:], esum[:], e[:])
                if esum is not None:
                    pending.append(_make_col(esum, cs, cj))
            _flush_pending()

            nc.sync.dma_start(out=o_st_ap[:], in_=st_ap[:])
            nc.sync.dma_start(out=o_st_aa[:], in_=st_aa[:])
            nc.sync.dma_start(out=o_st_pp[:], in_=st_pp[:])
            nc.sync.dma_start(out=o_diag[:], in_=diag[:])
    return nc


def combine(core_outs):
    """core_outs: list (per core) of dicts with the 7 output arrays.

    aa/pp self-terms are masked to zero on device, so no -2*exp(1/T)
    correction is needed here."""
    rs_ap = np.empty(N, np.float32)
    rs_aa = np.empty(N, np.float32)
    rs_pp = np.empty(N, np.float32)
    diag = np.empty(N, np.float32)
    cs_ap_tot = np.zeros(N, np.float64)
    aa_contrib = np.zeros(N, np.float64)
    pp_contrib = np.zeros(N, np.float64)

    for k, o in enumerate(core_outs):
        sl = slice(k * B, (k + 1) * B)
        # st[p, m*ng+g] -> local row m*128+p; sum over g
        rs_ap[sl] = o["st_ap"].reshape(128, MT, 8).sum(-1).T.reshape(B)
        rs_aa[sl] = o["st_aa"].reshape(128, MT, 5).sum(-1).T.reshape(B)
        rs_pp[sl] = o["st_pp"].reshape(128, MT, 5).sum(-1).T.reshape(B)
        diag[sl] = o["diag"].T.reshape(B)

        cs_ap_tot += np.roll(o["cs_ap"].reshape(N).astype(np.float64), k * B)
        for row, j in enumerate(SYM_COL_BLOCKS):
            v = np.zeros(N, np.float64)
            v[j * B:(j + 1) * B] = o["cs_aa"].reshape(3, B)[row]
            aa_contrib += np.roll(v, k * B)
            v = np.zeros(N, np.float64)
            v[j * B:(j + 1) * B] = o["cs_pp"].reshape(3, B)[row]
            pp_contrib += np.roll(v, k * B)

    partition = (rs_ap.astype(np.float64) + cs_ap_tot
                 + rs_aa.astype(np.float64) + aa_contrib
                 + rs_pp.astype(np.float64) + pp_contrib)
    pos_logit = INV_T * diag.astype(np.float64)
    loss = -(pos_logit - np.log(partition)).mean()
    return np.float32(loss)


def _split_waits(nc):
    """Walrus codegen allows ~1 sync wait per instruction; hoist extra
    waits onto same-engine NoOps inserted just before the instruction."""
    for fn in nc.m.functions:
        for blk in fn.blocks:
            new = []
            for inst in blk.instructions:
                si = getattr(inst, "sync_info", None)
                keep = 1
                if si is not None and si.on_wait and len(si.on_wait) > keep:
                    waits = list(si.on_wait)
                    for i, w in enumerate(waits[:-keep]):
                        nop = mybir.InstNoOp(name=f"{inst.name}-sw{i}")
                        nop.engine = inst.engine
                        nop.sync_info = mybir.SyncInfo(on_wait=[w], on_update=[])
                        new.append(nop)
                    inst.sync_info = mybir.SyncInfo(
                        on_wait=list(waits[-keep:]),
                        on_update=list(si.on_update))
                new.append(inst)
            blk.instructions = new


_NC_CACHE = None


def _get_program():
    global _NC_CACHE
    if _NC_CACHE is None:
        _NC_CACHE = build_program()
        _split_waits(_NC_CACHE)
    return _NC_CACHE


def run(anchor_embeddings, positive_embeddings, trace=False, **trace_kwargs):
    a = np.ascontiguousarray(anchor_embeddings, dtype=np.float32)
    p = np.ascontiguousarray(positive_embeddings, dtype=np.float32)
    in_maps = [
        {"a": np.roll(a, -k * B, axis=0), "p": np.roll(p, -k * B, axis=0)}
        for k in range(NCORES)
    ]
    nc = _get_program()
    res = run_bass_kernel_spmd(nc, in_maps, list(range(NCORES)), trace=trace,
                               **trace_kwargs)
    return combine(res.results), res


def kernel(anchor_embeddings, positive_embeddings):
    loss, _ = run(anchor_embeddings, positive_embeddings)
    return loss



# revision 34
# speedup vs baseline: 2.2172x; 1.0198x over previous
"""GTE contrastive loss kernel for 8 Trainium2 NeuronCores.

Math (reference): loss = -mean_i( cos(a_i,p_i)/T - log(partition_i) ),
partition_i = sum_j E_ap[i,j] + sum_j E_aa[i,j] + sum_j E_ap[j,i]
            + sum_j E_pp[j,i] - 2*exp(1/T),   E_xy = exp(cos/T).

Sharding: core k owns row block k (1024 rows).  Inputs are rotated by
-1024k rows per core so one SPMD program suffices: "my rows" are always
rows 0:1024 of the rotated input, and column block j means global block
(k+j) mod 8.  Symmetry of E_aa/E_pp lets each core compute only column
blocks 0..4: blocks 1..3 also emit column sums which cover the missing
row-sum pieces of blocks 5..7 on other cores; block 4 is computed by
both endpoint cores (rowsum only) so it is never double counted.
"""

import os
import sys

import numpy as np

for _p in ("/opt/trn_rl_repo", os.path.expanduser("/root/.axon_site/_ro/trn_rl_repo")):
    if os.path.isdir(_p) and _p not in sys.path:
        sys.path.insert(0, _p)

from concourse import bass, bass_isa, masks, tile  # noqa: E402
from concourse.bass_utils import run_bass_kernel_spmd  # noqa: E402

mybir = bass.mybir
F32 = mybir.dt.float32
F32R = mybir.dt.float32r
BF16 = mybir.dt.bfloat16

N, D, NCORES = 8192, 64, 8
B = N // NCORES            # 1024 rows per core
MT = B // 128              # 8 row tiles of 128
TFULL = N // 128           # 64 transpose tiles
NCHUNK = 4                 # norm/transpose pipeline chunks
CT = TFULL // NCHUNK       # t-tiles per chunk
CTOK = N // NCHUNK         # tokens per chunk
INV_T = 20.0

AP_BLOCKS = list(range(8))       # ap: all column blocks, all with colsum
SYM_BLOCKS = [0, 1, 2, 3, 4]     # aa/pp: cyclic half
SYM_COL_BLOCKS = [1, 2, 3]       # aa/pp blocks that also emit colsums


def _emit_load_normalize(nc, tc, ctx, dram_in, name):
    """Chunk-pipelined DMA [N,D] -> [128,64,64] + row-normalize (bf16)."""
    singles = ctx.enter_context(tc.tile_pool(name=f"{name}_sb", bufs=1))
    tmp = ctx.enter_context(tc.tile_pool(name=f"{name}_tmp", bufs=1))

    nat = singles.tile([128, TFULL, D], F32)
    nat2 = singles.tile([128, TFULL, D], BF16, tag=f"{name}_nat2")
    sq = tmp.tile([128, TFULL, D], F32, tag="sqtmp")
    ss = tmp.tile([128, TFULL], F32, tag="ss")
    nrm = tmp.tile([128, TFULL], F32, tag="nrm")
    inv = tmp.tile([128, TFULL], F32, tag="inv")
    src = dram_in[:].rearrange("(t p) d -> p t d", p=128)
    for h in range(NCHUNK):
        sl = slice(h * CT, (h + 1) * CT)
        nc.sync.dma_start(out=nat[:, sl, :], in_=src[:, sl, :])
        nc.vector.tensor_mul(sq[:, sl, :], nat[:, sl, :], nat[:, sl, :])
        nc.vector.tensor_reduce(ss[:, sl], sq[:, sl, :],
                                axis=mybir.AxisListType.X,
                                op=mybir.AluOpType.add)
        nc.scalar.activation(nrm[:, sl], ss[:, sl],
                             mybir.ActivationFunctionType.Sqrt)
        nc.vector.reciprocal(inv[:, sl], nrm[:, sl])
        inv_b = inv[:, sl].broadcast_to([128, CT, D])
        nc.vector.tensor_mul(nat2[:, sl, :], nat[:, sl, :], inv_b)
    return singles, nat2


def _emit_transpose(nc, tc, singles, nat, ident, name, nchunk=NCHUNK):
    """PE-transpose bf16 nat [128,64,64] -> nchunk xT tiles [64, CTOK].

    Chunked tiles keep dependency tracking fine-grained: the first jobs
    start as soon as chunk 0 is transposed.  nchunk < NCHUNK skips the
    tail tokens (aT tokens beyond block 4+own-rows are never read)."""
    xTl = [singles.tile([64, CTOK], BF16, tag=f"{name}_xT{h}", name=f"{name}_xT{h}")
           for h in range(nchunk)]
    with tc.tile_pool(name=f"{name}_tr", bufs=2, space="PSUM") as trp:
        for q in range(nchunk * CT // 4):
            h, qo = divmod(q, CT // 4)
            tr = trp.tile([64, 512], BF16, tag="tr")
            for s in range(4):
                t = q * 4 + s
                nc.tensor.transpose(tr[:, s * 128:(s + 1) * 128], nat[:, t, :],
                                    ident[:])
            nc.vector.tensor_copy(xTl[h][:, qo * 512:(qo + 1) * 512], tr[:])
    return xTl


def _mov(xTl, j, c):
    """Moving-operand slice for block j, 512-half c, from chunked xT."""
    tok = j * B + c * 512
    h, off = divmod(tok, CTOK)
    return xTl[h][:, off:off + 512]


def build_program():
    nc = bass.Bass()
    a_in = nc.declare_dram_parameter("a", [N, D], F32, isOutput=False)
    p_in = nc.declare_dram_parameter("p", [N, D], F32, isOutput=False)
    o_st_ap = nc.declare_dram_parameter("st_ap", [128, MT * 8], F32, isOutput=True)
    o_st_aa = nc.declare_dram_parameter("st_aa", [128, MT * 5], F32, isOutput=True)
    o_st_pp = nc.declare_dram_parameter("st_pp", [128, MT * 5], F32, isOutput=True)
    o_diag = nc.declare_dram_parameter("diag", [128, MT], F32, isOutput=True)
    o_cs_ap = nc.declare_dram_parameter("cs_ap", [1, 8 * B], F32, isOutput=True)
    o_cs_aa = nc.declare_dram_parameter("cs_aa", [1, 3 * B], F32, isOutput=True)
    o_cs_pp = nc.declare_dram_parameter("cs_pp", [1, 3 * B], F32, isOutput=True)

    with tile.TileContext(nc) as tc:
        import contextlib

        with contextlib.ExitStack() as ctx:
            a_sing, a_nat = _emit_load_normalize(nc, tc, ctx, a_in, "a")

            res = ctx.enter_context(tc.tile_pool(name="results", bufs=1))
            st_ap = res.tile([128, MT * 8], F32)
            st_aa = res.tile([128, MT * 5], F32)
            st_pp = res.tile([128, MT * 5], F32)
            diag = res.tile([128, MT], F32)

            ident0 = res.tile([128, 128], F32)
            masks.make_identity(nc, ident0[:])
            ident = res.tile([128, 128], BF16)
            nc.vector.tensor_copy(ident[:], ident0[:])
            # -100 on the diagonal: exp(20*(s-100)) == 0, so the aa/pp
            # self-terms drop out on device (no e^20 cancellation on host,
            # which reduced matmul precision cannot support)
            msk = res.tile([128, 128], F32)
            nc.vector.tensor_scalar_mul(msk[:], ident0[:], -100.0)

            aTl = _emit_transpose(nc, tc, a_sing, a_nat, ident, "a", nchunk=3)
            p_sing, p_nat = _emit_load_normalize(nc, tc, ctx, p_in, "p")

            # diagonal cos(a_i, p_i) for own rows (block 0 of rotated input)
            dtmp = res.tile([128, MT, D], F32)
            nc.vector.tensor_mul(dtmp[:], a_nat[:, 0:MT, :], p_nat[:, 0:MT, :])
            nc.vector.tensor_reduce(diag[:], dtmp[:], axis=mybir.AxisListType.X,
                                    op=mybir.AluOpType.add)

            ones_t = res.tile([128, 128], BF16)
            nc.vector.memset(ones_t[:], 1.0)
            csp = ctx.enter_context(tc.tile_pool(name="csstage", bufs=2))
            mmp = ctx.enter_context(tc.tile_pool(name="mm", bufs=2, space="PSUM"))
            colp = ctx.enter_context(tc.tile_pool(name="col", bufs=1, space="PSUM"))
            ep = ctx.enter_context(tc.tile_pool(name="etile", bufs=3))
            sump = ctx.enter_context(tc.tile_pool(name="esum", bufs=2))

            # aa jobs run first (they only need aT); pT transposes are
            # emitted after two aa jobs so the PE transpose burst overlaps
            # with scalar exp work instead of extending the prologue.
            jobs = []
            for g, j in enumerate(SYM_BLOCKS):
                cj = SYM_COL_BLOCKS.index(j) if j in SYM_COL_BLOCKS else None
                jobs.append(("aa", j, st_aa, g, 5, o_cs_aa, cj))
            for g, j in enumerate(AP_BLOCKS):
                jobs.append(("ap", j, st_ap, g, 8, o_cs_ap, j))
            for g, j in enumerate(SYM_BLOCKS):
                cj = SYM_COL_BLOCKS.index(j) if j in SYM_COL_BLOCKS else None
                jobs.append(("pp", j, st_pp, g, 5, o_cs_pp, cj))

            # Colsum work (2 matmuls on the DVE-accumulated e-sum) is
            # deferred so the in-order PE stream never waits on exp/DVE.
            pending = []

            def _flush_pending():
                while pending:
                    pending.pop(0)()

            def _make_col(esum, cs, cj):
                def emit():
                    col_ps = colp.tile([128, B], F32, tag="col")
                    for c in range(2):
                        nc.tensor.matmul(
                            col_ps[:, c * 512:(c + 1) * 512],
                            ones_t[:],
                            esum[:, c * 512:(c + 1) * 512],
                            start=True, stop=True,
                        )
                    # stage via DVE (not scalar: keep exp unblocked)
                    cstage = csp.tile([1, B], F32, tag="cs")
                    nc.vector.tensor_copy(cstage[:], col_ps[0:1, :])
                    nc.sync.dma_start(out=cs[0:1, cj * B:(cj + 1) * B],
                                      in_=cstage[:])
                return emit

            pTl = None
            for jn, (kind, j, st, g, ng, cs, cj) in enumerate(jobs):
                if (jn == 2 or kind != "aa") and pTl is None:
                    pTl = _emit_transpose(nc, tc, p_sing, p_nat, ident, "p")
                xTl = aTl if kind in ("aa", "ap") else pTl
                yTl = pTl if kind in ("ap", "pp") else aTl
                sym_diag = kind != "ap" and j == 0
                esum = None
                e_first = None
                if cj is not None:
                    esum = sump.tile([128, B], BF16, tag="esum")
                for m in range(MT):
                    mm_ps = mmp.tile([128, B], F32, tag="mm")
                    for c in range(2):
                        nc.tensor.matmul(
                            mm_ps[:, c * 512:(c + 1) * 512],
                            xTl[0][:, m * 128:(m + 1) * 128],
                            _mov(yTl, j, c),
                            start=True, stop=True,
                        )
                    if m == 1:
                        _flush_pending()
                    if sym_diag:
                        nc.vector.tensor_add(
                            mm_ps[:, m * 128:(m + 1) * 128],
                            mm_ps[:, m * 128:(m + 1) * 128], msk[:])
                    e = ep.tile([128, B], BF16, tag="e")
                    nc.scalar.activation(
                        e[:], mm_ps[:], mybir.ActivationFunctionType.Exp,
                        scale=INV_T,
                        accum_out=st[:, m * ng + g: m * ng + g + 1],
                    )
                    if esum is not None:
                        # running bf16 e-sum on DVE replaces per-tile PE
                        # colsum matmuls (8x fewer PE colsum rows)
                        if m == 0:
                            e_first = e
                        elif m == 1:
                            nc.vector.tensor_add(esum[:], e_first[:], e[:])
                        else:
                            nc.vector.tensor_add(esum[:], esum[:], e[:])
                if esum is not None:
                    pending.append(_make_col(esum, cs, cj))
            _flush_pending()

            nc.sync.dma_start(out=o_st_ap[:], in_=st_ap[:])
            nc.sync.dma_start(out=o_st_aa[:], in_=st_aa[:])
            nc.sync.dma_start(out=o_st_pp[:], in_=st_pp[:])
            nc.sync.dma_start(out=o_diag[:], in_=diag[:])
    return nc


def combine(core_outs):
    """core_outs: list (per core) of dicts with the 7 output arrays.

    aa/pp self-terms are masked to zero on device, so no -2*exp(1/T)
    correction is needed here."""
    rs_ap = np.empty(N, np.float32)
    rs_aa = np.empty(N, np.float32)
    rs_pp = np.empty(N, np.float32)
    diag = np.empty(N, np.float32)
    cs_ap_tot = np.zeros(N, np.float64)
    aa_contrib = np.zeros(N, np.float64)
    pp_contrib = np.zeros(N, np.float64)

    for k, o in enumerate(core_outs):
        sl = slice(k * B, (k + 1) * B)
        # st[p, m*ng+g] -> local row m*128+p; sum over g
        rs_ap[sl] = o["st_ap"].reshape(128, MT, 8).sum(-1).T.reshape(B)
        rs_aa[sl] = o["st_aa"].reshape(128, MT, 5).sum(-1).T.reshape(B)
        rs_pp[sl] = o["st_pp"].reshape(128, MT, 5).sum(-1).T.reshape(B)
        diag[sl] = o["diag"].T.reshape(B)

        cs_ap_tot += np.roll(o["cs_ap"].reshape(N).astype(np.float64), k * B)
        for row, j in enumerate(SYM_COL_BLOCKS):
            v = np.zeros(N, np.float64)
            v[j * B:(j + 1) * B] = o["cs_aa"].reshape(3, B)[row]
            aa_contrib += np.roll(v, k * B)
            v = np.zeros(N, np.float64)
            v[j * B:(j + 1) * B] = o["cs_pp"].reshape(3, B)[row]
            pp_contrib += np.roll(v, k * B)

    partition = (rs_ap.astype(np.float64) + cs_ap_tot
                 + rs_aa.astype(np.float64) + aa_contrib
                 + rs_pp.astype(np.float64) + pp_contrib)
    pos_logit = INV_T * diag.astype(np.float64)
    loss = -(pos_logit - np.log(partition)).mean()
    return np.float32(loss)


def _split_waits(nc):
    """Walrus codegen allows ~1 sync wait per instruction; hoist extra
    waits onto same-engine NoOps inserted just before the instruction."""
    for fn in nc.m.functions:
        for blk in fn.blocks:
            new = []
            for inst in blk.instructions:
                si = getattr(inst, "sync_info", None)
                keep = 1
                if si is not None and si.on_wait and len(si.on_wait) > keep:
                    waits = list(si.on_wait)
                    for i, w in enumerate(waits[:-keep]):
                        nop = mybir.InstNoOp(name=f"{inst.name}-sw{i}")
                        nop.engine = inst.engine
                        nop.sync_info = mybir.SyncInfo(on_wait=[w], on_update=[])
                        new.append(nop)
                    inst.sync_info = mybir.SyncInfo(
                        on_wait=list(waits[-keep:]),
                        on_update=list(si.on_update))
                new.append(inst)
            blk.instructions = new


_NC_CACHE = None


def _get_program():
    global _NC_CACHE
    if _NC_CACHE is None:
        _NC_CACHE = build_program()
        # populate .instr bytes for extended-ISA ops (partition_all_reduce)
        mybir.codegen_inst_isa_subclasses(_NC_CACHE)
        _split_waits(_NC_CACHE)
    return _NC_CACHE


def run(anchor_embeddings, positive_embeddings, trace=False, **trace_kwargs):
    a = np.ascontiguousarray(anchor_embeddings, dtype=np.float32)
    p = np.ascontiguousarray(positive_embeddings, dtype=np.float32)
    in_maps = [
        {"a": np.roll(a, -k * B, axis=0), "p": np.roll(p, -k * B, axis=0)}
        for k in range(NCORES)
    ]
    nc = _get_program()
    res = run_bass_kernel_spmd(nc, in_maps, list(range(NCORES)), trace=trace,
                               **trace_kwargs)
    return combine(res.results), res


def kernel(anchor_embeddings, positive_embeddings):
    loss, _ = run(anchor_embeddings, positive_embeddings)
    return loss



# revision 37
# speedup vs baseline: 2.2495x; 1.0146x over previous
"""GTE contrastive loss kernel for 8 Trainium2 NeuronCores.

Math (reference): loss = -mean_i( cos(a_i,p_i)/T - log(partition_i) ),
partition_i = sum_j E_ap[i,j] + sum_j E_aa[i,j] + sum_j E_ap[j,i]
            + sum_j E_pp[j,i] - 2*exp(1/T),   E_xy = exp(cos/T).

Sharding: core k owns row block k (1024 rows).  Inputs are rotated by
-1024k rows per core so one SPMD program suffices: "my rows" are always
rows 0:1024 of the rotated input, and column block j means global block
(k+j) mod 8.  Symmetry of E_aa/E_pp lets each core compute only column
blocks 0..4: blocks 1..3 also emit column sums which cover the missing
row-sum pieces of blocks 5..7 on other cores; block 4 is computed by
both endpoint cores (rowsum only) so it is never double counted.
"""

import os
import sys

import numpy as np

for _p in ("/opt/trn_rl_repo", os.path.expanduser("/root/.axon_site/_ro/trn_rl_repo")):
    if os.path.isdir(_p) and _p not in sys.path:
        sys.path.insert(0, _p)

from concourse import bass, bass_isa, masks, tile  # noqa: E402
from concourse.bass_utils import run_bass_kernel_spmd  # noqa: E402

mybir = bass.mybir
F32 = mybir.dt.float32
F32R = mybir.dt.float32r
BF16 = mybir.dt.bfloat16

N, D, NCORES = 8192, 64, 8
B = N // NCORES            # 1024 rows per core
MT = B // 128              # 8 row tiles of 128
TFULL = N // 128           # 64 transpose tiles
NCHUNK = 4                 # norm/transpose pipeline chunks
CT = TFULL // NCHUNK       # t-tiles per chunk
CTOK = N // NCHUNK         # tokens per chunk
INV_T = 20.0

AP_BLOCKS = list(range(8))       # ap: all column blocks, all with colsum
SYM_BLOCKS = [0, 1, 2, 3, 4]     # aa/pp: cyclic half
SYM_COL_BLOCKS = [1, 2, 3]       # aa/pp blocks that also emit colsums


def _emit_load_normalize(nc, tc, ctx, dram_in, name):
    """Chunk-pipelined DMA [N,D] -> [128,64,64] + row-normalize (bf16)."""
    singles = ctx.enter_context(tc.tile_pool(name=f"{name}_sb", bufs=1))
    tmp = ctx.enter_context(tc.tile_pool(name=f"{name}_tmp", bufs=1))

    nat = singles.tile([128, TFULL, D], F32)
    nat2 = singles.tile([128, TFULL, D], BF16, tag=f"{name}_nat2")
    sq = tmp.tile([128, TFULL, D], F32, tag="sqtmp")
    ss = tmp.tile([128, TFULL], F32, tag="ss")
    nrm = tmp.tile([128, TFULL], F32, tag="nrm")
    inv = tmp.tile([128, TFULL], F32, tag="inv")
    src = dram_in[:].rearrange("(t p) d -> p t d", p=128)
    for h in range(NCHUNK):
        sl = slice(h * CT, (h + 1) * CT)
        nc.sync.dma_start(out=nat[:, sl, :], in_=src[:, sl, :])
        nc.vector.tensor_mul(sq[:, sl, :], nat[:, sl, :], nat[:, sl, :])
        nc.vector.tensor_reduce(ss[:, sl], sq[:, sl, :],
                                axis=mybir.AxisListType.X,
                                op=mybir.AluOpType.add)
        nc.scalar.activation(nrm[:, sl], ss[:, sl],
                             mybir.ActivationFunctionType.Sqrt)
        nc.vector.reciprocal(inv[:, sl], nrm[:, sl])
        inv_b = inv[:, sl].broadcast_to([128, CT, D])
        nc.vector.tensor_mul(nat2[:, sl, :], nat[:, sl, :], inv_b)
    return singles, nat2


def _emit_transpose(nc, tc, singles, nat, ident, name, nchunk=NCHUNK):
    """PE-transpose bf16 nat [128,64,64] -> nchunk xT tiles [64, CTOK].

    Chunked tiles keep dependency tracking fine-grained: the first jobs
    start as soon as chunk 0 is transposed.  nchunk < NCHUNK skips the
    tail tokens (aT tokens beyond block 4+own-rows are never read)."""
    xTl = [singles.tile([64, CTOK], BF16, tag=f"{name}_xT{h}", name=f"{name}_xT{h}")
           for h in range(nchunk)]
    with tc.tile_pool(name=f"{name}_tr", bufs=2, space="PSUM") as trp:
        for q in range(nchunk * CT // 4):
            h, qo = divmod(q, CT // 4)
            tr = trp.tile([64, 512], BF16, tag="tr")
            for s in range(4):
                t = q * 4 + s
                nc.tensor.transpose(tr[:, s * 128:(s + 1) * 128], nat[:, t, :],
                                    ident[:])
            nc.vector.tensor_copy(xTl[h][:, qo * 512:(qo + 1) * 512], tr[:])
    return xTl


def _mov(xTl, j, c):
    """Moving-operand slice for block j, 512-half c, from chunked xT."""
    tok = j * B + c * 512
    h, off = divmod(tok, CTOK)
    return xTl[h][:, off:off + 512]


def build_program():
    nc = bass.Bass()
    a_in = nc.declare_dram_parameter("a", [N, D], F32, isOutput=False)
    p_in = nc.declare_dram_parameter("p", [N, D], F32, isOutput=False)
    o_st_ap = nc.declare_dram_parameter("st_ap", [128, MT * 8], F32, isOutput=True)
    o_st_aa = nc.declare_dram_parameter("st_aa", [128, MT * 5], F32, isOutput=True)
    o_st_pp = nc.declare_dram_parameter("st_pp", [128, MT * 5], F32, isOutput=True)
    o_diag = nc.declare_dram_parameter("diag", [128, MT], F32, isOutput=True)
    o_cs_ap = nc.declare_dram_parameter("cs_ap", [1, 8 * B], F32, isOutput=True)
    o_cs_aa = nc.declare_dram_parameter("cs_aa", [1, 3 * B], F32, isOutput=True)
    o_cs_pp = nc.declare_dram_parameter("cs_pp", [1, 3 * B], F32, isOutput=True)

    with tile.TileContext(nc) as tc:
        import contextlib

        with contextlib.ExitStack() as ctx:
            a_sing, a_nat = _emit_load_normalize(nc, tc, ctx, a_in, "a")

            res = ctx.enter_context(tc.tile_pool(name="results", bufs=1))
            st_ap = res.tile([128, MT * 8], F32)
            st_aa = res.tile([128, MT * 5], F32)
            st_pp = res.tile([128, MT * 5], F32)
            diag = res.tile([128, MT], F32)

            ident0 = res.tile([128, 128], F32)
            masks.make_identity(nc, ident0[:])
            ident = res.tile([128, 128], BF16)
            nc.vector.tensor_copy(ident[:], ident0[:])
            # -100 on the diagonal: exp(20*(s-100)) == 0, so the aa/pp
            # self-terms drop out on device (no e^20 cancellation on host,
            # which reduced matmul precision cannot support)
            msk = res.tile([128, 128], F32)
            nc.vector.tensor_scalar_mul(msk[:], ident0[:], -100.0)

            aTl = _emit_transpose(nc, tc, a_sing, a_nat, ident, "a", nchunk=3)

            ones_t = res.tile([128, 128], BF16)
            nc.vector.memset(ones_t[:], 1.0)
            csp = ctx.enter_context(tc.tile_pool(name="csstage", bufs=2))
            mmp = ctx.enter_context(tc.tile_pool(name="mm", bufs=2, space="PSUM"))
            colp = ctx.enter_context(tc.tile_pool(name="col", bufs=1, space="PSUM"))
            ep = ctx.enter_context(tc.tile_pool(name="etile", bufs=3))
            sump = ctx.enter_context(tc.tile_pool(name="esum", bufs=2))

            # aa jobs run first (they only need aT); pT transposes are
            # emitted after two aa jobs so the PE transpose burst overlaps
            # with scalar exp work instead of extending the prologue.
            jobs = []
            for g, j in enumerate(SYM_BLOCKS):
                cj = SYM_COL_BLOCKS.index(j) if j in SYM_COL_BLOCKS else None
                jobs.append(("aa", j, st_aa, g, 5, o_cs_aa, cj))
            for g, j in enumerate(AP_BLOCKS):
                jobs.append(("ap", j, st_ap, g, 8, o_cs_ap, j))
            for g, j in enumerate(SYM_BLOCKS):
                cj = SYM_COL_BLOCKS.index(j) if j in SYM_COL_BLOCKS else None
                jobs.append(("pp", j, st_pp, g, 5, o_cs_pp, cj))

            # Colsum work (2 matmuls on the DVE-accumulated e-sum) is
            # deferred so the in-order PE stream never waits on exp/DVE.
            pending = []

            def _flush_pending():
                while pending:
                    pending.pop(0)()

            def _make_col(esum, cs, cj):
                def emit():
                    col_ps = colp.tile([128, B], F32, tag="col")
                    for c in range(2):
                        nc.tensor.matmul(
                            col_ps[:, c * 512:(c + 1) * 512],
                            ones_t[:],
                            esum[:, c * 512:(c + 1) * 512],
                            start=True, stop=True,
                        )
                    # stage via DVE (not scalar: keep exp unblocked)
                    cstage = csp.tile([1, B], F32, tag="cs")
                    nc.vector.tensor_copy(cstage[:], col_ps[0:1, :])
                    nc.sync.dma_start(out=cs[0:1, cj * B:(cj + 1) * B],
                                      in_=cstage[:])
                return emit

            # p's load/norm/transposes and the ap-diagonal are emitted two
            # jobs in: the in-order scalar/DVE queues then run aa0/aa1 exps
            # first instead of blocking the first exp behind p's sqrts.
            pTl = None
            for jn, (kind, j, st, g, ng, cs, cj) in enumerate(jobs):
                if (jn == 2 or kind != "aa") and pTl is None:
                    p_sing, p_nat = _emit_load_normalize(nc, tc, ctx, p_in, "p")
                    pTl = _emit_transpose(nc, tc, p_sing, p_nat, ident, "p")
                    dtmp = res.tile([128, MT, D], F32)
                    nc.vector.tensor_mul(dtmp[:], a_nat[:, 0:MT, :],
                                         p_nat[:, 0:MT, :])
                    nc.vector.tensor_reduce(diag[:], dtmp[:],
                                            axis=mybir.AxisListType.X,
                                            op=mybir.AluOpType.add)
                xTl = aTl if kind in ("aa", "ap") else pTl
                yTl = pTl if kind in ("ap", "pp") else aTl
                sym_diag = kind != "ap" and j == 0
                esum = None
                e_first = None
                if cj is not None:
                    esum = sump.tile([128, B], BF16, tag="esum")
                for m in range(MT):
                    mm_ps = mmp.tile([128, B], F32, tag="mm")
                    for c in range(2):
                        nc.tensor.matmul(
                            mm_ps[:, c * 512:(c + 1) * 512],
                            xTl[0][:, m * 128:(m + 1) * 128],
                            _mov(yTl, j, c),
                            start=True, stop=True,
                        )
                    if m == 1:
                        _flush_pending()
                    if sym_diag:
                        nc.vector.tensor_add(
                            mm_ps[:, m * 128:(m + 1) * 128],
                            mm_ps[:, m * 128:(m + 1) * 128], msk[:])
                    e = ep.tile([128, B], BF16, tag="e")
                    nc.scalar.activation(
                        e[:], mm_ps[:], mybir.ActivationFunctionType.Exp,
                        scale=INV_T,
                        accum_out=st[:, m * ng + g: m * ng + g + 1],
                    )
                    if esum is not None:
                        # running bf16 e-sum on DVE replaces per-tile PE
                        # colsum matmuls (8x fewer PE colsum rows)
                        if m == 0:
                            e_first = e
                        elif m == 1:
                            nc.vector.tensor_add(esum[:], e_first[:], e[:])
                        else:
                            nc.vector.tensor_add(esum[:], esum[:], e[:])
                if esum is not None:
                    pending.append(_make_col(esum, cs, cj))
                if kind == "aa" and j == SYM_BLOCKS[-1]:
                    nc.sync.dma_start(out=o_st_aa[:], in_=st_aa[:])
                    nc.sync.dma_start(out=o_diag[:], in_=diag[:])
                elif kind == "ap" and j == AP_BLOCKS[-1]:
                    nc.sync.dma_start(out=o_st_ap[:], in_=st_ap[:])
            _flush_pending()

            nc.sync.dma_start(out=o_st_pp[:], in_=st_pp[:])
    return nc


def combine(core_outs):
    """core_outs: list (per core) of dicts with the 7 output arrays.

    aa/pp self-terms are masked to zero on device, so no -2*exp(1/T)
    correction is needed here."""
    rs_ap = np.empty(N, np.float32)
    rs_aa = np.empty(N, np.float32)
    rs_pp = np.empty(N, np.float32)
    diag = np.empty(N, np.float32)
    cs_ap_tot = np.zeros(N, np.float64)
    aa_contrib = np.zeros(N, np.float64)
    pp_contrib = np.zeros(N, np.float64)

    for k, o in enumerate(core_outs):
        sl = slice(k * B, (k + 1) * B)
        # st[p, m*ng+g] -> local row m*128+p; sum over g
        rs_ap[sl] = o["st_ap"].reshape(128, MT, 8).sum(-1).T.reshape(B)
        rs_aa[sl] = o["st_aa"].reshape(128, MT, 5).sum(-1).T.reshape(B)
        rs_pp[sl] = o["st_pp"].reshape(128, MT, 5).sum(-1).T.reshape(B)
        diag[sl] = o["diag"].T.reshape(B)

        cs_ap_tot += np.roll(o["cs_ap"].reshape(N).astype(np.float64), k * B)
        for row, j in enumerate(SYM_COL_BLOCKS):
            v = np.zeros(N, np.float64)
            v[j * B:(j + 1) * B] = o["cs_aa"].reshape(3, B)[row]
            aa_contrib += np.roll(v, k * B)
            v = np.zeros(N, np.float64)
            v[j * B:(j + 1) * B] = o["cs_pp"].reshape(3, B)[row]
            pp_contrib += np.roll(v, k * B)

    partition = (rs_ap.astype(np.float64) + cs_ap_tot
                 + rs_aa.astype(np.float64) + aa_contrib
                 + rs_pp.astype(np.float64) + pp_contrib)
    pos_logit = INV_T * diag.astype(np.float64)
    loss = -(pos_logit - np.log(partition)).mean()
    return np.float32(loss)


def _split_waits(nc):
    """Walrus codegen allows ~1 sync wait per instruction; hoist extra
    waits onto same-engine NoOps inserted just before the instruction."""
    for fn in nc.m.functions:
        for blk in fn.blocks:
            new = []
            for inst in blk.instructions:
                si = getattr(inst, "sync_info", None)
                keep = 1
                if si is not None and si.on_wait and len(si.on_wait) > keep:
                    waits = list(si.on_wait)
                    for i, w in enumerate(waits[:-keep]):
                        nop = mybir.InstNoOp(name=f"{inst.name}-sw{i}")
                        nop.engine = inst.engine
                        nop.sync_info = mybir.SyncInfo(on_wait=[w], on_update=[])
                        new.append(nop)
                    inst.sync_info = mybir.SyncInfo(
                        on_wait=list(waits[-keep:]),
                        on_update=list(si.on_update))
                new.append(inst)
            blk.instructions = new


_NC_CACHE = None


def _get_program():
    global _NC_CACHE
    if _NC_CACHE is None:
        _NC_CACHE = build_program()
        # populate .instr bytes for extended-ISA ops (partition_all_reduce)
        mybir.codegen_inst_isa_subclasses(_NC_CACHE)
        _split_waits(_NC_CACHE)
    return _NC_CACHE


def run(anchor_embeddings, positive_embeddings, trace=False, **trace_kwargs):
    a = np.ascontiguousarray(anchor_embeddings, dtype=np.float32)
    p = np.ascontiguousarray(positive_embeddings, dtype=np.float32)
    in_maps = [
        {"a": np.roll(a, -k * B, axis=0), "p": np.roll(p, -k * B, axis=0)}
        for k in range(NCORES)
    ]
    nc = _get_program()
    res = run_bass_kernel_spmd(nc, in_maps, list(range(NCORES)), trace=trace,
                               **trace_kwargs)
    return combine(res.results), res


def kernel(anchor_embeddings, positive_embeddings):
    loss, _ = run(anchor_embeddings, positive_embeddings)
    return loss



# revision 42
# speedup vs baseline: 2.3111x; 1.0274x over previous
"""GTE contrastive loss kernel for 8 Trainium2 NeuronCores.

Math (reference): loss = -mean_i( cos(a_i,p_i)/T - log(partition_i) ),
partition_i = sum_j E_ap[i,j] + sum_j E_aa[i,j] + sum_j E_ap[j,i]
            + sum_j E_pp[j,i] - 2*exp(1/T),   E_xy = exp(cos/T).

Sharding: core k owns row block k (1024 rows).  Inputs are rotated by
-1024k rows per core so one SPMD program suffices: "my rows" are always
rows 0:1024 of the rotated input, and column block j means global block
(k+j) mod 8.  Symmetry of E_aa/E_pp lets each core compute only column
blocks 0..4: blocks 1..3 also emit column sums which cover the missing
row-sum pieces of blocks 5..7 on other cores; block 4 is computed by
both endpoint cores (rowsum only) so it is never double counted.
"""

import os
import sys

import numpy as np

for _p in ("/opt/trn_rl_repo", os.path.expanduser("/root/.axon_site/_ro/trn_rl_repo")):
    if os.path.isdir(_p) and _p not in sys.path:
        sys.path.insert(0, _p)

from concourse import bass, bass_isa, masks, tile  # noqa: E402
from concourse.bass_utils import run_bass_kernel_spmd  # noqa: E402

mybir = bass.mybir
F32 = mybir.dt.float32
F32R = mybir.dt.float32r
BF16 = mybir.dt.bfloat16

N, D, NCORES = 8192, 64, 8
B = N // NCORES            # 1024 rows per core
MT = B // 128              # 8 row tiles of 128
TFULL = N // 128           # 64 transpose tiles
NCHUNK = 4                 # norm/transpose pipeline chunks
CT = TFULL // NCHUNK       # t-tiles per chunk
CTOK = N // NCHUNK         # tokens per chunk
INV_T = 20.0

AP_BLOCKS = list(range(8))       # ap: all column blocks, all with colsum
SYM_BLOCKS = [0, 1, 2, 3, 4]     # aa/pp: cyclic half
SYM_COL_BLOCKS = [1, 2, 3]       # aa/pp blocks that also emit colsums


def _mov(xTl, j, c):
    """Moving-operand slice for block j, 512-half c, from chunked xT."""
    tok = j * B + c * 512
    w = xTl[0].shape[1]
    h, off = divmod(tok, w)
    return xTl[h][:, off:off + 512]


def _transpose_tiles(nc, trp, nat, ident, xT, tstart, tcount):
    """PE-transpose nat tiles [tstart, tstart+tcount) into xT columns."""
    for q in range(tcount // 4):
        tr = trp.tile([64, 512], BF16, tag="tr")
        for s in range(4):
            t = tstart + q * 4 + s
            nc.tensor.transpose(tr[:, s * 128:(s + 1) * 128], nat[:, t, :],
                                ident[:])
        nc.vector.tensor_copy(xT[:, q * 512:(q + 1) * 512], tr[:])


def build_program():
    nc = bass.Bass()
    a_in = nc.declare_dram_parameter("a", [N, D], F32, isOutput=False)
    p_in = nc.declare_dram_parameter("p", [N, D], F32, isOutput=False)
    o_st_ap = nc.declare_dram_parameter("st_ap", [128, MT * 8], F32, isOutput=True)
    o_st_aa = nc.declare_dram_parameter("st_aa", [128, MT * 5], F32, isOutput=True)
    o_st_pp = nc.declare_dram_parameter("st_pp", [128, MT * 5], F32, isOutput=True)
    o_diag = nc.declare_dram_parameter("diag", [128, MT], F32, isOutput=True)
    o_cs_ap = nc.declare_dram_parameter("cs_ap", [1, 8 * B], F32, isOutput=True)
    o_cs_aa = nc.declare_dram_parameter("cs_aa", [1, 3 * B], F32, isOutput=True)
    o_cs_pp = nc.declare_dram_parameter("cs_pp", [1, 3 * B], F32, isOutput=True)

    with tile.TileContext(nc) as tc:
        import contextlib

        with contextlib.ExitStack() as ctx:
            res = ctx.enter_context(tc.tile_pool(name="results", bufs=1))
            st_ap = res.tile([128, MT * 8], F32)
            st_aa = res.tile([128, MT * 5], F32)
            st_pp = res.tile([128, MT * 5], F32)
            diag = res.tile([128, MT], F32)

            # setup first: nothing below depends on inputs, so the DVE/PE
            # queues are clear when the first data chunk lands
            ident0 = res.tile([128, 128], F32)
            masks.make_identity(nc, ident0[:])
            ident = res.tile([128, 128], BF16)
            nc.vector.tensor_copy(ident[:], ident0[:])
            # -100 on the diagonal: exp(20*(s-100)) == 0, so the aa/pp
            # self-terms drop out on device (no e^20 cancellation on host,
            # which reduced matmul precision cannot support)
            msk = res.tile([128, 128], F32)
            nc.vector.tensor_scalar_mul(msk[:], ident0[:], -100.0)
            ones_t = res.tile([128, 128], BF16)
            nc.vector.memset(ones_t[:], 1.0)

            csp = ctx.enter_context(tc.tile_pool(name="csstage", bufs=2))
            mmp = ctx.enter_context(tc.tile_pool(name="mm", bufs=2, space="PSUM"))
            colp = ctx.enter_context(tc.tile_pool(name="col", bufs=1, space="PSUM"))
            trp = ctx.enter_context(tc.tile_pool(name="tr", bufs=2, space="PSUM"))
            ep = ctx.enter_context(tc.tile_pool(name="etile", bufs=3))
            sump = ctx.enter_context(tc.tile_pool(name="esum", bufs=2))

            # --- a-side: 5 chunks of 1024 tokens (blocks 5..7 of aT are
            # never read), each norm+transpose chunk emitted just before
            # the aa job that first needs it ---
            ACH, ACT = 5, 8                       # chunks, t-tiles/chunk
            a_sb = ctx.enter_context(tc.tile_pool(name="a_sb", bufs=1))
            a_nat = a_sb.tile([128, ACH * ACT, D], F32)
            a_nat2 = a_sb.tile([128, ACH * ACT, D], BF16)
            a_ss = a_sb.tile([128, ACH * ACT], F32)
            a_nrm = a_sb.tile([128, ACH * ACT], F32)
            a_inv = a_sb.tile([128, ACH * ACT], F32)
            a_sq = a_sb.tile([128, ACH * ACT, D], F32)
            a_src = a_in[:].rearrange("(t p) d -> p t d", p=128)
            aTl = [a_sb.tile([64, B], BF16, name=f"a_xT{h}") for h in range(ACH)]

            def emit_a_chunk(h):
                sl = slice(h * ACT, (h + 1) * ACT)
                nc.sync.dma_start(out=a_nat[:, sl, :], in_=a_src[:, sl, :])
                nc.vector.tensor_mul(a_sq[:, sl, :], a_nat[:, sl, :],
                                     a_nat[:, sl, :])
                nc.vector.tensor_reduce(a_ss[:, sl], a_sq[:, sl, :],
                                        axis=mybir.AxisListType.X,
                                        op=mybir.AluOpType.add)
                nc.scalar.activation(a_nrm[:, sl], a_ss[:, sl],
                                     mybir.ActivationFunctionType.Sqrt)
                nc.vector.reciprocal(a_inv[:, sl], a_nrm[:, sl])
                inv_b = a_inv[:, sl].broadcast_to([128, ACT, D])
                nc.vector.tensor_mul(a_nat2[:, sl, :], a_nat[:, sl, :], inv_b)
                _transpose_tiles(nc, trp, a_nat2, ident, aTl[h], h * ACT, ACT)

            # aa jobs run first (they only need aT); pT transposes are
            # emitted after two aa jobs so the PE transpose burst overlaps
            # with scalar exp work instead of extending the prologue.
            jobs = []
            for g, j in enumerate(SYM_BLOCKS):
                cj = SYM_COL_BLOCKS.index(j) if j in SYM_COL_BLOCKS else None
                jobs.append(("aa", j, st_aa, g, 5, o_cs_aa, cj))
            for g, j in enumerate(AP_BLOCKS):
                jobs.append(("ap", j, st_ap, g, 8, o_cs_ap, j))
            for g, j in enumerate(SYM_BLOCKS):
                cj = SYM_COL_BLOCKS.index(j) if j in SYM_COL_BLOCKS else None
                jobs.append(("pp", j, st_pp, g, 5, o_cs_pp, cj))

            # Colsum work (2 matmuls on the DVE-accumulated e-sum) is
            # deferred so the in-order PE stream never waits on exp/DVE.
            pending = []

            def _flush_pending():
                while pending:
                    pending.pop(0)()

            def _make_col(esum, cs, cj):
                def emit():
                    col_ps = colp.tile([128, B], F32, tag="col")
                    for c in range(2):
                        nc.tensor.matmul(
                            col_ps[:, c * 512:(c + 1) * 512],
                            ones_t[:],
                            esum[:, c * 512:(c + 1) * 512],
                            start=True, stop=True,
                        )
                    # stage via DVE (not scalar: keep exp unblocked)
                    cstage = csp.tile([1, B], F32, tag="cs")
                    nc.vector.tensor_copy(cstage[:], col_ps[0:1, :])
                    nc.sync.dma_start(out=cs[0:1, cj * B:(cj + 1) * B],
                                      in_=cstage[:])
                return emit

            # --- p-side: emitted two jobs in, so the in-order scalar/DVE
            # queues run aa0/aa1 exps first; one sqrt for all chunks keeps
            # the mid-stream activation-table thrash to a single reload ---
            def emit_p():
                p_sb = ctx.enter_context(tc.tile_pool(name="p_sb", bufs=1))
                p_nat = p_sb.tile([128, TFULL, D], F32)
                p_nat2 = p_sb.tile([128, TFULL, D], BF16)
                p_sq = p_sb.tile([128, TFULL, D], F32)
                p_ss = p_sb.tile([128, TFULL], F32)
                p_nrm = p_sb.tile([128, TFULL], F32)
                p_inv = p_sb.tile([128, TFULL], F32)
                p_src = p_in[:].rearrange("(t p) d -> p t d", p=128)
                pTl = [p_sb.tile([64, CTOK], BF16, name=f"p_xT{h}")
                       for h in range(NCHUNK)]
                for h in range(NCHUNK):
                    sl = slice(h * CT, (h + 1) * CT)
                    nc.sync.dma_start(out=p_nat[:, sl, :], in_=p_src[:, sl, :])
                    nc.vector.tensor_mul(p_sq[:, sl, :], p_nat[:, sl, :],
                                         p_nat[:, sl, :])
                    nc.vector.tensor_reduce(p_ss[:, sl], p_sq[:, sl, :],
                                            axis=mybir.AxisListType.X,
                                            op=mybir.AluOpType.add)
                    nc.scalar.activation(p_nrm[:, sl], p_ss[:, sl],
                                         mybir.ActivationFunctionType.Sqrt)
                    nc.vector.reciprocal(p_inv[:, sl], p_nrm[:, sl])
                    inv_b = p_inv[:, sl].broadcast_to([128, CT, D])
                    nc.vector.tensor_mul(p_nat2[:, sl, :], p_nat[:, sl, :],
                                         inv_b)
                    _transpose_tiles(nc, trp, p_nat2, ident, pTl[h],
                                     h * CT, CT)
                # diagonal cos(a_i, p_i) for own rows (pos_logit)
                dtmp = res.tile([128, MT, D], F32)
                nc.vector.tensor_mul(dtmp[:], a_nat2[:, 0:MT, :],
                                     p_nat2[:, 0:MT, :])
                nc.vector.tensor_reduce(diag[:], dtmp[:],
                                        axis=mybir.AxisListType.X,
                                        op=mybir.AluOpType.add)
                return pTl

            for h in range(ACH):
                emit_a_chunk(h)

            pTl = None
            for jn, (kind, j, st, g, ng, cs, cj) in enumerate(jobs):
                if (jn == 2 or kind != "aa") and pTl is None:
                    pTl = emit_p()
                xTl = aTl if kind in ("aa", "ap") else pTl
                yTl = pTl if kind in ("ap", "pp") else aTl
                sym_diag = kind != "ap" and j == 0
                esum = None
                e_first = None
                if cj is not None:
                    esum = sump.tile([128, B], BF16, tag="esum")
                for m in range(MT):
                    mm_ps = mmp.tile([128, B], F32, tag="mm")
                    for c in range(2):
                        nc.tensor.matmul(
                            mm_ps[:, c * 512:(c + 1) * 512],
                            xTl[0][:, m * 128:(m + 1) * 128],
                            _mov(yTl, j, c),
                            start=True, stop=True,
                        )
                    if m == 1:
                        _flush_pending()
                    if sym_diag:
                        nc.vector.tensor_add(
                            mm_ps[:, m * 128:(m + 1) * 128],
                            mm_ps[:, m * 128:(m + 1) * 128], msk[:])
                    e = ep.tile([128, B], BF16, tag="e")
                    nc.scalar.activation(
                        e[:], mm_ps[:], mybir.ActivationFunctionType.Exp,
                        scale=INV_T,
                        accum_out=st[:, m * ng + g: m * ng + g + 1],
                    )
                    if esum is not None:
                        # running bf16 e-sum on DVE replaces per-tile PE
                        # colsum matmuls (8x fewer PE colsum rows)
                        if m == 0:
                            e_first = e
                        elif m == 1:
                            nc.vector.tensor_add(esum[:], e_first[:], e[:])
                        else:
                            nc.vector.tensor_add(esum[:], esum[:], e[:])
                if esum is not None:
                    pending.append(_make_col(esum, cs, cj))
                if kind == "aa" and j == SYM_BLOCKS[-1]:
                    nc.sync.dma_start(out=o_st_aa[:], in_=st_aa[:])
                    nc.sync.dma_start(out=o_diag[:], in_=diag[:])
                elif kind == "ap" and j == AP_BLOCKS[-1]:
                    nc.sync.dma_start(out=o_st_ap[:], in_=st_ap[:])
            _flush_pending()

            nc.sync.dma_start(out=o_st_pp[:], in_=st_pp[:])
    return nc


def combine(core_outs):
    """core_outs: list (per core) of dicts with the 7 output arrays.

    aa/pp self-terms are masked to zero on device, so no -2*exp(1/T)
    correction is needed here."""
    rs_ap = np.empty(N, np.float32)
    rs_aa = np.empty(N, np.float32)
    rs_pp = np.empty(N, np.float32)
    diag = np.empty(N, np.float32)
    cs_ap_tot = np.zeros(N, np.float64)
    aa_contrib = np.zeros(N, np.float64)
    pp_contrib = np.zeros(N, np.float64)

    for k, o in enumerate(core_outs):
        sl = slice(k * B, (k + 1) * B)
        # st[p, m*ng+g] -> local row m*128+p; sum over g
        rs_ap[sl] = o["st_ap"].reshape(128, MT, 8).sum(-1).T.reshape(B)
        rs_aa[sl] = o["st_aa"].reshape(128, MT, 5).sum(-1).T.reshape(B)
        rs_pp[sl] = o["st_pp"].reshape(128, MT, 5).sum(-1).T.reshape(B)
        diag[sl] = o["diag"].T.reshape(B)

        cs_ap_tot += np.roll(o["cs_ap"].reshape(N).astype(np.float64), k * B)
        for row, j in enumerate(SYM_COL_BLOCKS):
            v = np.zeros(N, np.float64)
            v[j * B:(j + 1) * B] = o["cs_aa"].reshape(3, B)[row]
            aa_contrib += np.roll(v, k * B)
            v = np.zeros(N, np.float64)
            v[j * B:(j + 1) * B] = o["cs_pp"].reshape(3, B)[row]
            pp_contrib += np.roll(v, k * B)

    partition = (rs_ap.astype(np.float64) + cs_ap_tot
                 + rs_aa.astype(np.float64) + aa_contrib
                 + rs_pp.astype(np.float64) + pp_contrib)
    pos_logit = INV_T * diag.astype(np.float64)
    loss = -(pos_logit - np.log(partition)).mean()
    return np.float32(loss)


def _split_waits(nc):
    """Walrus codegen allows ~1 sync wait per instruction; hoist extra
    waits onto same-engine NoOps inserted just before the instruction."""
    for fn in nc.m.functions:
        for blk in fn.blocks:
            new = []
            for inst in blk.instructions:
                si = getattr(inst, "sync_info", None)
                keep = 1
                if si is not None and si.on_wait and len(si.on_wait) > keep:
                    waits = list(si.on_wait)
                    for i, w in enumerate(waits[:-keep]):
                        nop = mybir.InstNoOp(name=f"{inst.name}-sw{i}")
                        nop.engine = inst.engine
                        nop.sync_info = mybir.SyncInfo(on_wait=[w], on_update=[])
                        new.append(nop)
                    inst.sync_info = mybir.SyncInfo(
                        on_wait=list(waits[-keep:]),
                        on_update=list(si.on_update))
                new.append(inst)
            blk.instructions = new


_NC_CACHE = None


def _get_program():
    global _NC_CACHE
    if _NC_CACHE is None:
        _NC_CACHE = build_program()
        # populate .instr bytes for extended-ISA ops (partition_all_reduce)
        mybir.codegen_inst_isa_subclasses(_NC_CACHE)
        _split_waits(_NC_CACHE)
    return _NC_CACHE


def run(anchor_embeddings, positive_embeddings, trace=False, **trace_kwargs):
    a = np.ascontiguousarray(anchor_embeddings, dtype=np.float32)
    p = np.ascontiguousarray(positive_embeddings, dtype=np.float32)
    in_maps = [
        {"a": np.roll(a, -k * B, axis=0), "p": np.roll(p, -k * B, axis=0)}
        for k in range(NCORES)
    ]
    nc = _get_program()
    res = run_bass_kernel_spmd(nc, in_maps, list(range(NCORES)), trace=trace,
                               **trace_kwargs)
    return combine(res.results), res


def kernel(anchor_embeddings, positive_embeddings):
    loss, _ = run(anchor_embeddings, positive_embeddings)
    return loss



# revision 44
# speedup vs baseline: 2.3148x; 1.0016x over previous
"""GTE contrastive loss kernel for 8 Trainium2 NeuronCores.

Math (reference): loss = -mean_i( cos(a_i,p_i)/T - log(partition_i) ),
partition_i = sum_j E_ap[i,j] + sum_j E_aa[i,j] + sum_j E_ap[j,i]
            + sum_j E_pp[j,i] - 2*exp(1/T),   E_xy = exp(cos/T).

Sharding: core k owns row block k (1024 rows).  Inputs are rotated by
-1024k rows per core so one SPMD program suffices: "my rows" are always
rows 0:1024 of the rotated input, and column block j means global block
(k+j) mod 8.  Symmetry of E_aa/E_pp lets each core compute only column
blocks 0..4: blocks 1..3 also emit column sums which cover the missing
row-sum pieces of blocks 5..7 on other cores; block 4 is computed by
both endpoint cores (rowsum only) so it is never double counted.
"""

import os
import sys

import numpy as np

for _p in ("/opt/trn_rl_repo", os.path.expanduser("/root/.axon_site/_ro/trn_rl_repo")):
    if os.path.isdir(_p) and _p not in sys.path:
        sys.path.insert(0, _p)

from concourse import bass, bass_isa, masks, tile  # noqa: E402
from concourse.bass_utils import run_bass_kernel_spmd  # noqa: E402

mybir = bass.mybir
F32 = mybir.dt.float32
F32R = mybir.dt.float32r
BF16 = mybir.dt.bfloat16

N, D, NCORES = 8192, 64, 8
B = N // NCORES            # 1024 rows per core
MT = B // 128              # 8 row tiles of 128
TFULL = N // 128           # 64 transpose tiles
NCHUNK = 4                 # norm/transpose pipeline chunks
CT = TFULL // NCHUNK       # t-tiles per chunk
CTOK = N // NCHUNK         # tokens per chunk
INV_T = 20.0

AP_BLOCKS = list(range(8))       # ap: all column blocks, all with colsum
SYM_BLOCKS = [0, 1, 2, 3, 4]     # aa/pp: cyclic half
SYM_COL_BLOCKS = [1, 2, 3]       # aa/pp blocks that also emit colsums


def _mov(xTl, j, c):
    """Moving-operand slice for block j, 512-half c, from chunked xT."""
    tok = j * B + c * 512
    w = xTl[0].shape[1]
    h, off = divmod(tok, w)
    return xTl[h][:, off:off + 512]


def _transpose_tiles(nc, trp, nat, ident, xT, tstart, tcount):
    """PE-transpose nat tiles [tstart, tstart+tcount) into xT columns."""
    for q in range(tcount // 4):
        tr = trp.tile([64, 512], BF16, tag="tr")
        for s in range(4):
            t = tstart + q * 4 + s
            nc.tensor.transpose(tr[:, s * 128:(s + 1) * 128], nat[:, t, :],
                                ident[:])
        nc.vector.tensor_copy(xT[:, q * 512:(q + 1) * 512], tr[:])


def build_program():
    nc = bass.Bass()
    a_in = nc.declare_dram_parameter("a", [N, D], F32, isOutput=False)
    p_in = nc.declare_dram_parameter("p", [N, D], F32, isOutput=False)
    o_st_ap = nc.declare_dram_parameter("st_ap", [128, MT * 8], F32, isOutput=True)
    o_st_aa = nc.declare_dram_parameter("st_aa", [128, MT * 5], F32, isOutput=True)
    o_st_pp = nc.declare_dram_parameter("st_pp", [128, MT * 5], F32, isOutput=True)
    o_diag = nc.declare_dram_parameter("diag", [128, MT], F32, isOutput=True)
    o_cs_ap = nc.declare_dram_parameter("cs_ap", [1, 8 * B], F32, isOutput=True)
    o_cs_aa = nc.declare_dram_parameter("cs_aa", [1, 3 * B], F32, isOutput=True)
    o_cs_pp = nc.declare_dram_parameter("cs_pp", [1, 3 * B], F32, isOutput=True)

    with tile.TileContext(nc) as tc:
        import contextlib

        with contextlib.ExitStack() as ctx:
            res = ctx.enter_context(tc.tile_pool(name="results", bufs=1))
            st_ap = res.tile([128, MT * 8], F32)
            st_aa = res.tile([128, MT * 5], F32)
            st_pp = res.tile([128, MT * 5], F32)
            diag = res.tile([128, MT], F32)

            # setup first: nothing below depends on inputs, so the DVE/PE
            # queues are clear when the first data chunk lands
            ident0 = res.tile([128, 128], F32)
            masks.make_identity(nc, ident0[:])
            ident = res.tile([128, 128], BF16)
            nc.vector.tensor_copy(ident[:], ident0[:])
            # -100 on the diagonal: exp(20*(s-100)) == 0, so the aa/pp
            # self-terms drop out on device (no e^20 cancellation on host,
            # which reduced matmul precision cannot support)
            msk = res.tile([128, 128], F32)
            nc.vector.tensor_scalar_mul(msk[:], ident0[:], -100.0)
            ones_t = res.tile([128, 128], BF16)
            nc.vector.memset(ones_t[:], 1.0)

            csp = ctx.enter_context(tc.tile_pool(name="csstage", bufs=2))
            mmp = ctx.enter_context(tc.tile_pool(name="mm", bufs=2, space="PSUM"))
            colp = ctx.enter_context(tc.tile_pool(name="col", bufs=1, space="PSUM"))
            trp = ctx.enter_context(tc.tile_pool(name="tr", bufs=2, space="PSUM"))
            ep = ctx.enter_context(tc.tile_pool(name="etile", bufs=3))
            sump = ctx.enter_context(tc.tile_pool(name="esum", bufs=2))

            # --- a-side: 5 chunks of 1024 tokens (blocks 5..7 of aT are
            # never read), each norm+transpose chunk emitted just before
            # the aa job that first needs it ---
            ACH, ACT = 5, 8                       # chunks, t-tiles/chunk
            a_sb = ctx.enter_context(tc.tile_pool(name="a_sb", bufs=1))
            a_nat = a_sb.tile([128, ACH * ACT, D], F32)
            a_nat2 = a_sb.tile([128, ACH * ACT, D], BF16)
            a_ss = a_sb.tile([128, ACH * ACT], F32)
            a_nrm = a_sb.tile([128, ACH * ACT], F32)
            a_inv = a_sb.tile([128, ACH * ACT], F32)
            a_sq = a_sb.tile([128, ACH * ACT, D], F32)
            a_src = a_in[:].rearrange("(t p) d -> p t d", p=128)
            aTl = [a_sb.tile([64, B], BF16, name=f"a_xT{h}") for h in range(ACH)]

            def emit_a_chunk(h):
                sl = slice(h * ACT, (h + 1) * ACT)
                nc.sync.dma_start(out=a_nat[:, sl, :], in_=a_src[:, sl, :])
                nc.vector.tensor_mul(a_sq[:, sl, :], a_nat[:, sl, :],
                                     a_nat[:, sl, :])
                nc.vector.tensor_reduce(a_ss[:, sl], a_sq[:, sl, :],
                                        axis=mybir.AxisListType.X,
                                        op=mybir.AluOpType.add)
                nc.scalar.activation(a_nrm[:, sl], a_ss[:, sl],
                                     mybir.ActivationFunctionType.Sqrt)
                nc.vector.reciprocal(a_inv[:, sl], a_nrm[:, sl])
                inv_b = a_inv[:, sl].broadcast_to([128, ACT, D])
                nc.vector.tensor_mul(a_nat2[:, sl, :], a_nat[:, sl, :], inv_b)
                _transpose_tiles(nc, trp, a_nat2, ident, aTl[h], h * ACT, ACT)

            # aa jobs run first (they only need aT); pT transposes are
            # emitted after two aa jobs so the PE transpose burst overlaps
            # with scalar exp work instead of extending the prologue.
            jobs = []
            for g, j in enumerate(SYM_BLOCKS):
                cj = SYM_COL_BLOCKS.index(j) if j in SYM_COL_BLOCKS else None
                jobs.append(("aa", j, st_aa, g, 5, o_cs_aa, cj))
            for g, j in enumerate(AP_BLOCKS):
                jobs.append(("ap", j, st_ap, g, 8, o_cs_ap, j))
            for g, j in enumerate(SYM_BLOCKS):
                cj = SYM_COL_BLOCKS.index(j) if j in SYM_COL_BLOCKS else None
                jobs.append(("pp", j, st_pp, g, 5, o_cs_pp, cj))

            # Colsum work (2 matmuls on the DVE-accumulated e-sum) is
            # deferred so the in-order PE stream never waits on exp/DVE.
            pending = []

            def _flush_pending():
                while pending:
                    pending.pop(0)()

            def _make_col(esum, cs, cj):
                def emit():
                    col_ps = colp.tile([128, B], F32, tag="col")
                    for c in range(2):
                        nc.tensor.matmul(
                            col_ps[:, c * 512:(c + 1) * 512],
                            ones_t[:],
                            esum[:, c * 512:(c + 1) * 512],
                            start=True, stop=True,
                        )
                    # stage via DVE (not scalar: keep exp unblocked)
                    cstage = csp.tile([1, B], F32, tag="cs")
                    nc.vector.tensor_copy(cstage[:], col_ps[0:1, :])
                    nc.sync.dma_start(out=cs[0:1, cj * B:(cj + 1) * B],
                                      in_=cstage[:])
                return emit

            # --- p-side: emitted two jobs in, so the in-order scalar/DVE
            # queues run aa0/aa1 exps first; one sqrt for all chunks keeps
            # the mid-stream activation-table thrash to a single reload ---
            def emit_p():
                p_sb = ctx.enter_context(tc.tile_pool(name="p_sb", bufs=1))
                p_nat = p_sb.tile([128, TFULL, D], F32)
                p_nat2 = p_sb.tile([128, TFULL, D], BF16)
                p_sq = p_sb.tile([128, TFULL, D], F32)
                p_ss = p_sb.tile([128, TFULL], F32)
                p_nrm = p_sb.tile([128, TFULL], F32)
                p_inv = p_sb.tile([128, TFULL], F32)
                p_src = p_in[:].rearrange("(t p) d -> p t d", p=128)
                pTl = [p_sb.tile([64, CTOK], BF16, name=f"p_xT{h}")
                       for h in range(NCHUNK)]
                for h in range(NCHUNK):
                    sl = slice(h * CT, (h + 1) * CT)
                    nc.sync.dma_start(out=p_nat[:, sl, :], in_=p_src[:, sl, :])
                    nc.vector.tensor_mul(p_sq[:, sl, :], p_nat[:, sl, :],
                                         p_nat[:, sl, :])
                    nc.vector.tensor_reduce(p_ss[:, sl], p_sq[:, sl, :],
                                            axis=mybir.AxisListType.X,
                                            op=mybir.AluOpType.add)
                    nc.scalar.activation(p_nrm[:, sl], p_ss[:, sl],
                                         mybir.ActivationFunctionType.Sqrt)
                    nc.vector.reciprocal(p_inv[:, sl], p_nrm[:, sl])
                    inv_b = p_inv[:, sl].broadcast_to([128, CT, D])
                    nc.vector.tensor_mul(p_nat2[:, sl, :], p_nat[:, sl, :],
                                         inv_b)
                    _transpose_tiles(nc, trp, p_nat2, ident, pTl[h],
                                     h * CT, CT)
                # diagonal cos(a_i, p_i) for own rows (pos_logit)
                dtmp = res.tile([128, MT, D], F32)
                nc.vector.tensor_mul(dtmp[:], a_nat2[:, 0:MT, :],
                                     p_nat2[:, 0:MT, :])
                nc.vector.tensor_reduce(diag[:], dtmp[:],
                                        axis=mybir.AxisListType.X,
                                        op=mybir.AluOpType.add)
                return pTl

            for h in range(ACH):
                emit_a_chunk(h)

            pTl = None
            for jn, (kind, j, st, g, ng, cs, cj) in enumerate(jobs):
                if (jn == 2 or kind != "aa") and pTl is None:
                    pTl = emit_p()
                xTl = aTl if kind in ("aa", "ap") else pTl
                yTl = pTl if kind in ("ap", "pp") else aTl
                sym_diag = kind != "ap" and j == 0
                esum = None
                e_first = None
                if cj is not None:
                    esum = sump.tile([128, B], BF16, tag="esum")
                for m in range(MT):
                    mm_ps = mmp.tile([128, B], F32, tag="mm")
                    for c in range(2):
                        nc.tensor.matmul(
                            mm_ps[:, c * 512:(c + 1) * 512],
                            xTl[0][:, m * 128:(m + 1) * 128],
                            _mov(yTl, j, c),
                            start=True, stop=True,
                        )
                    if m == 1:
                        _flush_pending()
                    if sym_diag:
                        nc.vector.tensor_add(
                            mm_ps[:, m * 128:(m + 1) * 128],
                            mm_ps[:, m * 128:(m + 1) * 128], msk[:])
                    e = ep.tile([128, B], BF16, tag="e")
                    nc.scalar.activation(
                        e[:], mm_ps[:], mybir.ActivationFunctionType.Exp,
                        scale=INV_T,
                        accum_out=st[:, m * ng + g: m * ng + g + 1],
                    )
                    if esum is not None:
                        # running bf16 e-sum on DVE replaces per-tile PE
                        # colsum matmuls (8x fewer PE colsum rows)
                        if m == 0:
                            e_first = e
                        elif m == 1:
                            nc.vector.tensor_add(esum[:], e_first[:], e[:])
                        else:
                            nc.vector.tensor_add(esum[:], esum[:], e[:])
                if esum is not None:
                    pending.append(_make_col(esum, cs, cj))
                if kind == "aa" and j == SYM_BLOCKS[-1]:
                    nc.sync.dma_start(out=o_st_aa[:], in_=st_aa[:])
                    nc.sync.dma_start(out=o_diag[:], in_=diag[:])
                elif kind == "ap" and j == AP_BLOCKS[-1]:
                    nc.sync.dma_start(out=o_st_ap[:], in_=st_ap[:])
            _flush_pending()

            nc.sync.dma_start(out=o_st_pp[:], in_=st_pp[:])
    return nc


def combine(core_outs):
    """core_outs: list (per core) of dicts with the 7 output arrays.

    aa/pp self-terms are masked to zero on device, so no -2*exp(1/T)
    correction is needed here."""
    rs_ap = np.empty(N, np.float32)
    rs_aa = np.empty(N, np.float32)
    rs_pp = np.empty(N, np.float32)
    diag = np.empty(N, np.float32)
    cs_ap_tot = np.zeros(N, np.float64)
    aa_contrib = np.zeros(N, np.float64)
    pp_contrib = np.zeros(N, np.float64)

    for k, o in enumerate(core_outs):
        sl = slice(k * B, (k + 1) * B)
        # st[p, m*ng+g] -> local row m*128+p; sum over g
        rs_ap[sl] = o["st_ap"].reshape(128, MT, 8).sum(-1).T.reshape(B)
        rs_aa[sl] = o["st_aa"].reshape(128, MT, 5).sum(-1).T.reshape(B)
        rs_pp[sl] = o["st_pp"].reshape(128, MT, 5).sum(-1).T.reshape(B)
        diag[sl] = o["diag"].T.reshape(B)

        cs_ap_tot += np.roll(o["cs_ap"].reshape(N).astype(np.float64), k * B)
        for row, j in enumerate(SYM_COL_BLOCKS):
            v = np.zeros(N, np.float64)
            v[j * B:(j + 1) * B] = o["cs_aa"].reshape(3, B)[row]
            aa_contrib += np.roll(v, k * B)
            v = np.zeros(N, np.float64)
            v[j * B:(j + 1) * B] = o["cs_pp"].reshape(3, B)[row]
            pp_contrib += np.roll(v, k * B)

    partition = (rs_ap.astype(np.float64) + cs_ap_tot
                 + rs_aa.astype(np.float64) + aa_contrib
                 + rs_pp.astype(np.float64) + pp_contrib)
    pos_logit = INV_T * diag.astype(np.float64)
    loss = -(pos_logit - np.log(partition)).mean()
    return np.float32(loss)


def _split_waits(nc):
    """Walrus codegen allows ~1 sync wait per instruction; hoist extra
    waits onto same-engine NoOps inserted just before the instruction."""
    for fn in nc.m.functions:
        for blk in fn.blocks:
            new = []
            for inst in blk.instructions:
                si = getattr(inst, "sync_info", None)
                keep = 1
                if si is not None and si.on_wait and len(si.on_wait) > keep:
                    waits = list(si.on_wait)
                    for i, w in enumerate(waits[:-keep]):
                        nop = mybir.InstNoOp(name=f"{inst.name}-sw{i}")
                        nop.engine = inst.engine
                        nop.sync_info = mybir.SyncInfo(on_wait=[w], on_update=[])
                        new.append(nop)
                    inst.sync_info = mybir.SyncInfo(
                        on_wait=list(waits[-keep:]),
                        on_update=list(si.on_update))
                new.append(inst)
            blk.instructions = new


_NC_CACHE = None


def _get_program():
    global _NC_CACHE
    if _NC_CACHE is None:
        _NC_CACHE = build_program()
        # populate .instr bytes for extended-ISA ops (partition_all_reduce)
        mybir.codegen_inst_isa_subclasses(_NC_CACHE)
        _split_waits(_NC_CACHE)
    return _NC_CACHE


def run(anchor_embeddings, positive_embeddings, trace=False, **trace_kwargs):
    a = np.ascontiguousarray(anchor_embeddings, dtype=np.float32)
    p = np.ascontiguousarray(positive_embeddings, dtype=np.float32)
    in_maps = [
        {"a": np.roll(a, -k * B, axis=0), "p": np.roll(p, -k * B, axis=0)}
        for k in range(NCORES)
    ]
    nc = _get_program()
    res = run_bass_kernel_spmd(nc, in_maps, list(range(NCORES)), trace=trace,
                               **trace_kwargs)
    return combine(res.results), res


def kernel(anchor_embeddings, positive_embeddings):
    loss, _ = run(anchor_embeddings, positive_embeddings)
    return loss

